# revision 1
# baseline (speedup 1.0000x reference)
"""AFNO transformer block on 8 Trainium2 NeuronCores.

Distribution:
  Phase 1 (channel-block sharded): core k owns channels [96k, 96k+96).
    LN1 stats partial sums -> per-batch AllReduce -> LN1 apply, then the
    whole spectral path (rFFT2 as DFT matmuls, block-diagonal complex MLP,
    inverse rFFT2) entirely core-local. DFTs use "flip" matmuls (data as
    the stationary operand) so every stage lands in the layout the next
    stage contracts over - no on-chip transposes.
  AllToAll: filter output reshard (channel-sharded -> token-sharded).
  Phase 2 (token sharded): core j owns tokens [4050j, 4050j+4050).
    h = filt + LN1(x) + x assembled channel-major, LN2 folded into fc1
    (extended contraction row + PE select/broadcast of per-token scales
    via a host-passed one-hot mask), fc1 -> exact Gelu -> fc2 -> residual
    -> PE transpose -> token-major output.
"""
import math
import numpy as np

import concourse.bass as bass
import concourse.mybir as mybir
import concourse.tile as tile
from concourse import bacc
from concourse.bass_utils import run_bass_kernel_spmd

F32 = mybir.dt.float32
AF = mybir.ActivationFunctionType
OP = mybir.AluOpType
AX = mybir.AxisListType

NCORES = 8
B, H, W, C = 2, 90, 180, 768
BS = 96           # channels per core / AFNO block size
KW = 46           # kept W-frequency modes
HID = 3072
LAM = 0.01
EPS = 1e-5
TOK = B * H * W   # 32400
TSH = TOK // NCORES  # 4050
NM = KW * H       # modes per batch elem: 4140
SQN = math.sqrt(H * W)

TTS = [256] * 15 + [210]
TT0 = [sum(TTS[:i]) for i in range(len(TTS))]
NCC = 6    # 768/128
NMO = 24   # 3072/128


def _dft_consts():
    wv = np.arange(W, dtype=np.float64)[:, None]
    wf = np.arange(KW, dtype=np.float64)[None, :]
    ang = 2.0 * np.pi * wv * wf / W
    fwr = np.cos(ang) / math.sqrt(W)
    fwi = -np.sin(ang) / math.sqrt(W)
    fwpack = np.concatenate([fwr, fwi], axis=1)          # (180, 92)
    hv = np.arange(H, dtype=np.float64)[:, None]
    hf = np.arange(H, dtype=np.float64)[None, :]
    angh = 2.0 * np.pi * hv * hf / H
    fhc = np.cos(angh) / math.sqrt(H)                    # symmetric
    fhs = np.sin(angh) / math.sqrt(H)
    alpha = np.ones(KW); alpha[1:] = 2.0
    iwr = alpha[None, :] * np.cos(ang) / math.sqrt(W)    # (180, 46)
    iwi = -alpha[None, :] * np.sin(ang) / math.sqrt(W)
    f32 = np.float32
    return (fwpack[:90].astype(f32), fwpack[90:].astype(f32),
            fhc.astype(f32), fhs.astype(f32), (-fhs).astype(f32),
            np.ascontiguousarray(iwr.T).astype(f32),
            np.ascontiguousarray(iwi.T).astype(f32))


def _shard_pieces(jq):
    """(ha,hb,wa,wb,tok_off) pieces of within-batch shard jq."""
    s0, e0 = TSH * jq, TSH * jq + TSH
    pieces, t = [], s0
    while t < e0:
        h = t // W
        wa = t - h * W
        if wa != 0 or e0 - t < W:
            wb = min(W, wa + (e0 - t))
            pieces.append((h, h + 1, wa, wb, t - s0))
            t += wb - wa
        else:
            hb = min(H, h + (e0 - t) // W)
            pieces.append((h, hb, 0, W, t - s0))
            t += (hb - h) * W
    return pieces


_CACHE = {}


def _build_nc():
    if "nc" in _CACHE:
        return _CACHE["nc"]
    nc = bacc.Bacc("TRN2", target_bir_lowering=False, debug=False,
                   num_devices=NCORES)
    g = lambda n, s: nc.dram_tensor(n, s, F32, kind="ExternalInput")
    xw = g("xw", [W, B, H, BS])
    xc = g("xc", [C, TSH])
    fwp0 = g("fwp0", [90, 92]); fwp1 = g("fwp1", [90, 92])
    fhc = g("fhc", [90, 90]); fhs = g("fhs", [90, 90]); fhsm = g("fhsm", [90, 90])
    iwrt = g("iwrt", [KW, W]); iwit = g("iwit", [KW, W])
    w1r = g("w1r", [BS, BS]); w1i = g("w1i", [BS, BS]); w1im = g("w1im", [BS, BS])
    b1r = g("b1r", [BS, 1]); b1i = g("b1i", [BS, 1])
    w2a = g("w2a", [BS + 1, BS]); w2b = g("w2b", [BS + 1, BS])
    w2c = g("w2c", [BS + 1, BS]); w2d = g("w2d", [BS + 1, BS])
    g1col = g("g1col", [BS, 1]); spike = g("spike", [BS, 1])
    b2rr = g("b2rr", [1, BS]); b2ir = g("b2ir", [1, BS])
    fc1m = g("fc1m", [C, HID])
    uneg = g("uneg", [1, HID])
    gbias = g("gbias", [128, NMO])
    fc2w = g("fc2w", [HID, C])
    fc2b = g("fc2b", [128, NCC])
    g1f = g("g1f", [128, NCC]); be1f = g("be1f", [128, NCC])
    ones1 = g("ones1", [1, 128])
    ones128 = g("ones128", [128, 1])
    onesrow = g("onesrow", [1, 2 * NM])
    ident = g("ident", [128, 128])
    mask128 = g("mask128", [NCORES, 128])   # one-hot row = this core's shard

    out = nc.dram_tensor("out", [TSH, C], F32, kind="ExternalOutput")
    rg = [list(range(NCORES))]

    from contextlib import ExitStack
    with tile.TileContext(nc) as tc:
        with ExitStack() as _st0:
            cp = _st0.enter_context(tc.tile_pool(name="const", bufs=1))
            dram = _st0.enter_context(tc.tile_pool(name="dram", bufs=1, space="DRAM"))
            def cl(t, shape):
                nm = f"c_{t.name if hasattr(t, 'name') else t[:].tensor.name}"
                s = cp.tile(shape, F32, name=nm, tag=nm)
                nc.gpsimd.dma_start(s[:], t[:])
                return s
            c_fwp0 = cl(fwp0, [90, 92]); c_fwp1 = cl(fwp1, [90, 92])
            c_fhc = cl(fhc, [90, 90]); c_fhs = cl(fhs, [90, 90])
            c_fhsm = cl(fhsm, [90, 90])
            c_iwrt = cl(iwrt, [KW, W]); c_iwit = cl(iwit, [KW, W])
            c_w1r = cl(w1r, [BS, BS]); c_w1i = cl(w1i, [BS, BS])
            c_w1im = cl(w1im, [BS, BS])
            c_b1r = cl(b1r, [BS, 1]); c_b1i = cl(b1i, [BS, 1])
            c_w2a = cl(w2a, [BS + 1, BS]); c_w2b = cl(w2b, [BS + 1, BS])
            c_w2c = cl(w2c, [BS + 1, BS]); c_w2d = cl(w2d, [BS + 1, BS])
            c_g1col = cl(g1col, [BS, 1]); c_spike = cl(spike, [BS, 1])
            c_b2rr = cl(b2rr, [1, BS]); c_b2ir = cl(b2ir, [1, BS])
            c_ones1 = cl(ones1, [1, 128]); c_ones128 = cl(ones128, [128, 1])
            c_ident = cl(ident, [128, 128])
            c_gbias = cl(gbias, [128, NMO]); c_fc2b = cl(fc2b, [128, NCC])
            c_g1f = cl(g1f, [128, NCC]); c_be1f = cl(be1f, [128, NCC])
            c_uneg = cl(uneg, [1, HID])
            c_mask = cl(mask128, [NCORES, 128])
            c_eps = cp.tile([128, 1], F32, name="c_eps")
            nc.vector.memset(c_eps[:], EPS)

            a2a_in = dram.tile([NCORES, BS, TSH], F32)
            a2a_out = dram.tile([NCORES, BS, TSH], F32)
            st_in = [dram.tile([2, W, H], F32, name=f"st_in{b_}") for b_ in range(B)]
            st_out = [dram.tile([2, W, H], F32, name=f"st_out{b_}") for b_ in range(B)]
            s2d = dram.tile([B, H, BS, W], F32, name="s2d")

            # ================= phase 1 =================
            with ExitStack() as _st1:
                stp = _st1.enter_context(tc.tile_pool(name="stats", bufs=1))
                zp = _st1.enter_context(tc.tile_pool(name="zp", bufs=3))
                spA = _st1.enter_context(tc.tile_pool(name="spA", bufs=1))
                spB = _st1.enter_context(tc.tile_pool(name="spB", bufs=1))
                spQ = _st1.enter_context(tc.tile_pool(name="spQ", bufs=1))
                s2s = _st1.enter_context(tc.tile_pool(name="s2s", bufs=4))
                clp = _st1.enter_context(tc.tile_pool(name="clipp", bufs=2))
                pp = _st1.enter_context(tc.tile_pool(name="psum1", bufs=8, space="PSUM"))

                s_sum = [stp.tile([90, 2, H], F32, tag=f"ss{b_}", name=f"ssum{b_}") for b_ in range(B)]
                s_sq = [stp.tile([90, 2, H], F32, tag=f"sq{b_}", name=f"ssq{b_}") for b_ in range(B)]
                s_m = [stp.tile([90, 2, H], F32, tag=f"sm{b_}", name=f"sm{b_}") for b_ in range(B)]
                s_r = [stp.tile([90, 2, H], F32, tag=f"sr{b_}", name=f"sr{b_}") for b_ in range(B)]
                s_t = stp.tile([90, H], F32, tag="st_tmp", name="s_tmp")

                def load_zh(b, wc, ch):
                    zh = zp.tile([90, H, 48], F32, tag="z", name="zh")
                    nc.gpsimd.dma_start(
                        zh[:], xw[wc * 90:(wc + 1) * 90, b, :, ch * 48:(ch + 1) * 48])
                    return zh

                # ---- stats pass (both b) + AllReduce per b
                def _stk(t, kind):
                    return bass.AP(tensor=t[:].tensor,
                                   offset=t[:].offset + kind * W * H,
                                   ap=[[90, 90], [8100, 2], [1, 90]])

                for b in range(B):
                    for wc in range(2):
                        for ch in range(2):
                            zh = load_zh(b, wc, ch)
                            sqh = spQ.tile([90, H, 48], F32, tag="QU", name="sqh")
                            nc.scalar.activation(out=sqh[:], in_=zh[:], func=AF.Square)
                            if ch == 0:
                                nc.vector.reduce_sum(s_sum[b][:, wc, :], zh[:], axis=AX.X)
                                nc.vector.reduce_sum(s_sq[b][:, wc, :], sqh[:], axis=AX.X)
                            else:
                                nc.vector.reduce_sum(s_t[:], zh[:], axis=AX.X)
                                nc.vector.tensor_add(s_sum[b][:, wc, :], s_sum[b][:, wc, :], s_t[:])
                                nc.vector.reduce_sum(s_t[:], sqh[:], axis=AX.X)
                                nc.vector.tensor_add(s_sq[b][:, wc, :], s_sq[b][:, wc, :], s_t[:])
                    nc.sync.dma_start(_stk(st_in[b], 0), s_sum[b][:])
                    nc.sync.dma_start(_stk(st_in[b], 1), s_sq[b][:])
                    nc.gpsimd.collective_compute(
                        "AllReduce", OP.add, replica_groups=rg,
                        ins=[st_in[b][:].opt()], outs=[st_out[b][:].opt()])

                for b in range(B):
                    nc.sync.dma_start(s_sum[b][:], _stk(st_out[b], 0))
                    nc.sync.dma_start(s_sq[b][:], _stk(st_out[b], 1))
                    nc.vector.tensor_scalar(out=s_m[b][:], in0=s_sum[b][:],
                                            scalar1=1.0 / C, scalar2=None,
                                            op0=OP.mult)
                    nc.vector.tensor_scalar(out=s_r[b][:], in0=s_sq[b][:],
                                            scalar1=1.0 / C, scalar2=None,
                                            op0=OP.mult)
                    tmp = stp.tile([90, 2, H], F32, tag=f"tmp{b}", name=f"tmpb{b}")
                    nc.vector.tensor_mul(tmp[:], s_m[b][:], s_m[b][:])
                    nc.vector.tensor_sub(s_r[b][:], s_r[b][:], tmp[:])
                    nc.scalar.activation(out=s_r[b][:], in_=s_r[b][:],
                                         func=AF.Sqrt, bias=c_eps[:90])
                    nc.vector.reciprocal(s_r[b][:], s_r[b][:])

                for b in range(B):
                    # ---- reload z halves, LN1, F1
                    yb = spA.tile([90, 92, BS], F32, tag="YO", name="yb")
                    for ch in range(2):
                        zh0 = load_zh(b, 0, ch)
                        zh1 = load_zh(b, 1, ch)
                        for wc, zt in ((0, zh0), (1, zh1)):
                            for h in range(H):
                                nc.vector.tensor_scalar(
                                    out=zt[:, h, :], in0=zt[:, h, :],
                                    scalar1=s_m[b][:, wc, h:h + 1],
                                    scalar2=s_r[b][:, wc, h:h + 1],
                                    op0=OP.subtract, op1=OP.mult)
                        for cl_ in range(48):
                            c = ch * 48 + cl_
                            ps = pp.tile([90, 92], F32, tag="pp", name="psf1")
                            nc.tensor.matmul(ps[:], zh0[:, :, cl_], c_fwp0[:],
                                             start=True, stop=False)
                            nc.tensor.matmul(ps[:], zh1[:, :, cl_], c_fwp1[:],
                                             start=False, stop=True)
                            if c % 2 == 0:
                                nc.scalar.activation(out=yb[:, :, c], in_=ps[:],
                                                     func=AF.Copy)
                            else:
                                nc.vector.tensor_copy(yb[:, :, c], ps[:])

                    # ---- F2
                    zb = spB.tile([BS, 2, KW, H], F32, tag="ZO", name="zbt")
                    for wf in range(KW):
                        yr = yb[:, wf, :]
                        yi = yb[:, 46 + wf, :]
                        pr = pp.tile([BS, H], F32, tag="pp", name="psf2r")
                        nc.tensor.matmul(pr[:], yr, c_fhc[:], start=True, stop=False)
                        nc.tensor.matmul(pr[:], yi, c_fhs[:], start=False, stop=True)
                        pi = pp.tile([BS, H], F32, tag="pp", name="psf2i")
                        nc.tensor.matmul(pi[:], yi, c_fhc[:], start=True, stop=False)
                        nc.tensor.matmul(pi[:], yr, c_fhsm[:], start=False, stop=True)
                        nc.scalar.activation(out=zb[:, 0, wf, :], in_=pr[:],
                                             func=AF.Copy, scale=c_g1col[:])
                        nc.scalar.activation(out=zb[:, 1, wf, :], in_=pi[:],
                                             func=AF.Copy, scale=c_g1col[:])
                    nc.vector.tensor_scalar(out=zb[:, 0, 0, 0:1],
                                            in0=zb[:, 0, 0, 0:1],
                                            scalar1=c_spike[:], scalar2=None,
                                            op0=OP.add)

                    # ---- block MLP layer 1
                    o1 = spA.tile([BS + 1, 2, NM], F32, tag="YO", name="o1t")
                    zr_f = zb[:, 0].rearrange("p a b -> p (a b)")
                    zi_f = zb[:, 1].rearrange("p a b -> p (a b)")
                    n0 = 0
                    while n0 < NM:
                        nn_ = min(512, NM - n0)
                        zr_s = zr_f[:, n0:n0 + nn_]
                        zi_s = zi_f[:, n0:n0 + nn_]
                        por = pp.tile([BS, 512], F32, tag="pp", name="pso1r")
                        nc.tensor.matmul(por[:, :nn_], c_w1r[:], zr_s,
                                         start=True, stop=False)
                        nc.tensor.matmul(por[:, :nn_], c_w1im[:], zi_s,
                                         start=False, stop=True)
                        poi = pp.tile([BS, 512], F32, tag="pp", name="pso1i")
                        nc.tensor.matmul(poi[:, :nn_], c_w1i[:], zr_s,
                                         start=True, stop=False)
                        nc.tensor.matmul(poi[:, :nn_], c_w1r[:], zi_s,
                                         start=False, stop=True)
                        nc.scalar.activation(out=o1[0:BS, 0, n0:n0 + nn_],
                                             in_=por[:, :nn_], func=AF.Relu,
                                             bias=c_b1r[:])
                        nc.scalar.activation(out=o1[0:BS, 1, n0:n0 + nn_],
                                             in_=poi[:, :nn_], func=AF.Relu,
                                             bias=c_b1i[:])
                        n0 += nn_

                    # ---- block MLP layer 2 + softshrink
                    o2 = spB.tile([H, 2, KW, BS], F32, tag="ZO", name="o2t")
                    o1r_f = o1[:, 0]
                    o1i_f = o1[:, 1]
                    for wf in range(KW):
                        lr = o1r_f[0:BS, wf * H:(wf + 1) * H]
                        li = o1i_f[0:BS, wf * H:(wf + 1) * H]
                        pr = pp.tile([H, BS], F32, tag="pp", name="pso2r")
                        nc.tensor.matmul(pr[:], lr, c_w2a[0:BS, :], start=True, stop=False)
                        nc.tensor.matmul(pr[:], li, c_w2b[0:BS, :], start=False, stop=False)
                        nc.tensor.matmul(pr[:], c_ones1[:, 0:H], c_b2rr[:], start=False, stop=True)
                        pi = pp.tile([H, BS], F32, tag="pp", name="pso2i")
                        nc.tensor.matmul(pi[:], li, c_w2c[0:BS, :], start=True, stop=False)
                        nc.tensor.matmul(pi[:], lr, c_w2d[0:BS, :], start=False, stop=False)
                        nc.tensor.matmul(pi[:], c_ones1[:, 0:H], c_b2ir[:], start=False, stop=True)
                        for ri, psm in ((0, pr), (1, pi)):
                            clip = clp.tile([H, BS], F32, tag="clip", name="clipt")
                            nc.vector.tensor_scalar(out=clip[:], in0=psm[:],
                                                    scalar1=-LAM, scalar2=LAM,
                                                    op0=OP.max, op1=OP.min)
                            nc.vector.tensor_sub(o2[:, ri, wf, :], psm[:], clip[:])

                    # ---- inverse H-DFT -> u2r/u2i [46, (c, h)]
                    u2r = spQ.tile([KW, BS, H], F32, tag="QU", name="u2rt")
                    u2i = spA.tile([KW, BS, H], F32, tag="YO", name="u2it")
                    for c in range(BS):
                        lr = o2[:, 0, :, c]
                        li = o2[:, 1, :, c]
                        pur = pp.tile([KW, H], F32, tag="pp", name="psur")
                        nc.tensor.matmul(pur[:], lr, c_fhc[:], start=True, stop=False)
                        nc.tensor.matmul(pur[:], li, c_fhsm[:], start=False, stop=True)
                        pui = pp.tile([KW, H], F32, tag="pp", name="psui")
                        nc.tensor.matmul(pui[:], li, c_fhc[:], start=True, stop=False)
                        nc.tensor.matmul(pui[:], lr, c_fhs[:], start=False, stop=True)
                        if c % 2 == 0:
                            nc.scalar.activation(out=u2r[:, c, :], in_=pur[:],
                                                 func=AF.Copy)
                            nc.vector.tensor_copy(u2i[:, c, :], pui[:])
                        else:
                            nc.vector.tensor_copy(u2r[:, c, :], pur[:])
                            nc.scalar.activation(out=u2i[:, c, :], in_=pui[:],
                                                 func=AF.Copy)

                    # ---- inverse W-DFT -> DRAM bounce s2d
                    for c in range(BS):
                        pf = pp.tile([H, W], F32, tag="pp", name="psf")
                        nc.tensor.matmul(pf[:], u2r[:, c, :], c_iwrt[:],
                                         start=True, stop=False)
                        nc.tensor.matmul(pf[:], u2i[:, c, :], c_iwit[:],
                                         start=False, stop=True)
                        s2t = s2s.tile([H, W], F32, tag="s2t", name="s2t")
                        if c % 2 == 0:
                            nc.scalar.activation(out=s2t[:], in_=pf[:], func=AF.Copy)
                        else:
                            nc.vector.tensor_copy(s2t[:], pf[:])
                        nc.sync.dma_start(s2d[b, :, c, :], s2t[:])

                    # ---- a2a send pieces (DRAM -> DRAM)
                    for jq in range(4):
                        j = b * 4 + jq
                        for (ha, hb_, wa, wb_, toff) in _shard_pieces(jq):
                            src = s2d[b, ha:hb_, :, wa:wb_]
                            dst = bass.AP(
                                tensor=a2a_in[:].tensor,
                                offset=a2a_in[:].offset + (j * BS * TSH + toff),
                                ap=[[wb_ - wa, hb_ - ha], [TSH, BS], [1, wb_ - wa]])
                            nc.sync.dma_start(dst, src)

            nc.gpsimd.collective_compute(
                "AllToAll", OP.bypass, replica_groups=rg,
                ins=[a2a_in[:].opt()], outs=[a2a_out[:].opt()])

            # ================= phase 2 =================
            with ExitStack() as _st2:
                fc1p = _st2.enter_context(tc.tile_pool(name="fc1p", bufs=1))
                xcp = _st2.enter_context(tc.tile_pool(name="xcp", bufs=2))
                t1p = _st2.enter_context(tc.tile_pool(name="t1p", bufs=2))
                htokp = _st2.enter_context(tc.tile_pool(name="htokp", bufs=7))
                hidp = _st2.enter_context(tc.tile_pool(name="hidp", bufs=1))
                rowp = _st2.enter_context(tc.tile_pool(name="rowp", bufs=1))
                rw2 = _st2.enter_context(tc.tile_pool(name="rw2", bufs=1))
                bcp = _st2.enter_context(tc.tile_pool(name="bcp", bufs=1))
                fc2p = _st2.enter_context(tc.tile_pool(name="fc2p", bufs=2))
                outp = _st2.enter_context(tc.tile_pool(name="outp", bufs=1))
                ph = _st2.enter_context(tc.tile_pool(name="ph", bufs=2, space="PSUM"))
                po = _st2.enter_context(tc.tile_pool(name="po", bufs=2, space="PSUM"))
                pst = _st2.enter_context(tc.tile_pool(name="pst", bufs=2, space="PSUM"))
                pbc = _st2.enter_context(tc.tile_pool(name="pbc", bufs=1, space="PSUM"))
                ptr = _st2.enter_context(tc.tile_pool(name="ptr", bufs=1, space="PSUM"))
                c_fc1 = [fc1p.tile([128, HID], F32, tag=f"fc1_{i}", name=f"cfc1_{i}")
                         for i in range(NCC)]
                for i in range(NCC):
                    nc.gpsimd.dma_start(c_fc1[i][:], fc1m[i * 128:(i + 1) * 128, :])

                # all-shard LN1 stats rows [8, 4050], computed in place
                r1_8 = rowp.tile([NCORES, TSH], F32, tag="r18", name="r18")
                mr1_8 = rowp.tile([NCORES, TSH], F32, tag="mr18", name="mr18")
                rtmp = rowp.tile([NCORES, TSH], F32, tag="rtmp", name="rtmp")
                for kind, dstt in ((0, rtmp), (1, r1_8)):
                    for s in range(NCORES):
                        bb, jq = s // 4, s % 4
                        for (ha, hb_, wa, wb_, toff) in _shard_pieces(jq):
                            src_ = bass.AP(
                                tensor=st_out[bb][:].tensor,
                                offset=st_out[bb][:].offset
                                + (kind * W * H + wa * H + ha),
                                ap=[[0, 1], [1, hb_ - ha], [H, wb_ - wa]])
                            nc.sync.dma_start(
                                dstt[s:s + 1,
                                     toff:toff + (hb_ - ha) * (wb_ - wa)],
                                src_)
                nc.vector.tensor_scalar(out=mr1_8[:], in0=rtmp[:],
                                        scalar1=1.0 / C, scalar2=None,
                                        op0=OP.mult)           # m1
                nc.vector.tensor_scalar(out=r1_8[:], in0=r1_8[:],
                                        scalar1=1.0 / C, scalar2=None,
                                        op0=OP.mult)           # q/C
                nc.vector.tensor_mul(rtmp[:], mr1_8[:], mr1_8[:])
                nc.vector.tensor_sub(r1_8[:], r1_8[:], rtmp[:])  # var
                nc.scalar.activation(out=r1_8[:], in_=r1_8[:], func=AF.Sqrt,
                                     bias=c_eps[:NCORES])
                nc.vector.reciprocal(r1_8[:], r1_8[:])           # r1
                nc.vector.tensor_mul(mr1_8[:], mr1_8[:], r1_8[:])  # m1*r1

                for it, T in enumerate(TTS):
                    t0 = TT0[it]
                    # select+broadcast this core's r1 / m1*r1 rows
                    r1b = bcp.tile([128, 256], F32, tag="r1b")
                    mr1b = bcp.tile([128, 256], F32, tag="mr1b")
                    for rows, bt in ((r1_8, r1b), (mr1_8, mr1b)):
                        pb = pbc.tile([128, 256], F32, tag="pbc")
                        nc.tensor.matmul(pb[:, :T], c_mask[:], rows[:, t0:t0 + T],
                                         start=True, stop=True)
                        nc.scalar.activation(out=bt[:, :T], in_=pb[:, :T],
                                             func=AF.Copy)

                    # assemble htok per c-chunk
                    htoks = []
                    for cc in range(NCC):
                        xct = xcp.tile([128, 256], F32, tag="xct")
                        nc.gpsimd.dma_start(xct[:, :T],
                                            xc[cc * 128:(cc + 1) * 128, t0:t0 + T])
                        ht = htokp.tile([128, 256], F32, tag="htok")
                        htoks.append(ht)
                        # recv: global c rows cc*128..cc*128+128 from a2a_out
                        c0 = cc * 128
                        r0 = 0
                        while r0 < 128:
                            s_blk = (c0 + r0) // BS
                            c_in = (c0 + r0) % BS
                            nrow = min(BS - c_in, 128 - r0)
                            nc.gpsimd.dma_start(
                                ht[r0:r0 + nrow, :T],
                                a2a_out[s_blk, c_in:c_in + nrow, t0:t0 + T])
                            r0 += nrow
                        t1 = t1p.tile([128, 256], F32, tag="t1")
                        nc.vector.tensor_mul(t1[:, :T], xct[:, :T], r1b[:, :T])
                        nc.vector.tensor_sub(t1[:, :T], t1[:, :T], mr1b[:, :T])
                        nc.vector.tensor_scalar(out=t1[:, :T], in0=t1[:, :T],
                                                scalar1=c_g1f[:, cc:cc + 1],
                                                scalar2=c_be1f[:, cc:cc + 1],
                                                op0=OP.mult, op1=OP.add)
                        nc.vector.tensor_add(ht[:, :T], ht[:, :T], xct[:, :T])
                        nc.vector.tensor_add(ht[:, :T], ht[:, :T], t1[:, :T])

                    # LN2 stats via ones-matmul
                    ps_s = pst.tile([1, 256], F32, tag="pst")
                    ps_q = pst.tile([1, 256], F32, tag="pst")
                    for cc in range(NCC):
                        nc.tensor.matmul(ps_s[:, :T], c_ones128[:], htoks[cc][:, :T],
                                         start=(cc == 0), stop=(cc == NCC - 1))
                    for cc in range(NCC):
                        hsq = t1p.tile([128, 256], F32, tag="t1", name="hsq")
                        nc.vector.tensor_mul(hsq[:, :T], htoks[cc][:, :T],
                                             htoks[cc][:, :T])
                        nc.tensor.matmul(ps_q[:, :T], c_ones128[:], hsq[:, :T],
                                         start=(cc == 0), stop=(cc == NCC - 1))
                    m2r = rw2.tile([1, 256], F32, tag="m2r")
                    r2r = rw2.tile([1, 256], F32, tag="r2r")
                    vv = rw2.tile([1, 256], F32, tag="vv")
                    nc.vector.tensor_scalar(out=m2r[:, :T], in0=ps_s[:, :T],
                                            scalar1=1.0 / C, scalar2=None,
                                            op0=OP.mult)
                    nc.vector.tensor_scalar(out=r2r[:, :T], in0=ps_q[:, :T],
                                            scalar1=1.0 / C, scalar2=None,
                                            op0=OP.mult)
                    nc.vector.tensor_mul(vv[:, :T], m2r[:, :T], m2r[:, :T])
                    nc.vector.tensor_sub(r2r[:, :T], r2r[:, :T], vv[:, :T])
                    nc.scalar.activation(out=r2r[:, :T], in_=r2r[:, :T],
                                         func=AF.Sqrt, bias=c_eps[:1])
                    nc.vector.reciprocal(r2r[:, :T], r2r[:, :T])
                    r2b = bcp.tile([128, 256], F32, tag="r2b")
                    pb = pbc.tile([128, 256], F32, tag="pbc")
                    nc.tensor.matmul(pb[:, :T], c_ones1[:], r2r[:, :T],
                                     start=True, stop=True)
                    nc.scalar.activation(out=r2b[:, :T], in_=pb[:, :T], func=AF.Copy)

                    # fc1 + LN2 fold + gelu
                    hid = hidp.tile([128, NMO, 256], F32, tag="hid")
                    for mo in range(NMO):
                        php = ph.tile([128, 256], F32, tag="ph")
                        for cc in range(NCC):
                            nc.tensor.matmul(
                                php[:, :T],
                                c_fc1[cc][:, mo * 128:(mo + 1) * 128],
                                htoks[cc][:, :T],
                                start=(cc == 0), stop=False)
                        nc.tensor.matmul(php[:, :T],
                                         c_uneg[:, mo * 128:(mo + 1) * 128],
                                         m2r[:, :T], start=False, stop=True)
                        t2 = t1p.tile([128, 256], F32, tag="t2")
                        nc.vector.tensor_mul(t2[:, :T], php[:, :T], r2b[:, :T])
                        nc.scalar.activation(out=hid[:, mo, :T], in_=t2[:, :T],
                                             func=AF.Gelu,
                                             bias=c_gbias[:, mo:mo + 1])

                    # fc2 (stream [128,128] weight pieces) + bias + residual
                    for co in range(NCC):
                        pop = po.tile([128, 256], F32, tag="po")
                        for ho in range(NMO):
                            wt = fc2p.tile([128, 128], F32, tag="fc2w")
                            nc.gpsimd.dma_start(
                                wt[:], fc2w[ho * 128:(ho + 1) * 128,
                                            co * 128:(co + 1) * 128])
                            nc.tensor.matmul(
                                pop[:, :T], wt[:], hid[:, ho, :T],
                                start=(ho == 0), stop=(ho == NMO - 1))
                        nc.vector.scalar_tensor_tensor(
                            out=htoks[co][:, :T], in0=pop[:, :T],
                            scalar=c_fc2b[:, co:co + 1], in1=htoks[co][:, :T],
                            op0=OP.add, op1=OP.add)

                    # transpose to token-major and store
                    nsub = (T + 127) // 128
                    for sub in range(nsub):
                        ns = min(128, T - sub * 128)
                        osb = outp.tile([128, C], F32, tag="osb")
                        for co in range(NCC):
                            pt = ptr.tile([128, 128], F32, tag="ptr")
                            nc.tensor.transpose(
                                pt[:ns, :],
                                htoks[co][:, sub * 128:sub * 128 + ns],
                                c_ident[:])
                            if co % 2 == 0:
                                nc.scalar.activation(
                                    out=osb[:ns, co * 128:(co + 1) * 128],
                                    in_=pt[:ns, :], func=AF.Copy)
                            else:
                                nc.vector.tensor_copy(
                                    osb[:ns, co * 128:(co + 1) * 128], pt[:ns, :])
                        nc.sync.dma_start(
                            out[t0 + sub * 128: t0 + sub * 128 + ns, :],
                            osb[:ns, :])

    nc.compile()
    _CACHE["nc"] = nc
    return nc


def _host_prep(inputs):
    x = np.ascontiguousarray(np.asarray(inputs["x"], dtype=np.float32))
    g1 = np.asarray(inputs["g1"], np.float32); be1 = np.asarray(inputs["be1"], np.float32)
    g2 = np.asarray(inputs["g2"], np.float32); be2 = np.asarray(inputs["be2"], np.float32)
    w1 = np.asarray(inputs["w1"], np.float32); b1 = np.asarray(inputs["b1"], np.float32)
    w2 = np.asarray(inputs["w2"], np.float32); b2 = np.asarray(inputs["b2"], np.float32)
    fc1_w = np.asarray(inputs["fc1_w"], np.float32)
    fc1_b = np.asarray(inputs["fc1_b"], np.float32)
    fc2_w = np.asarray(inputs["fc2_w"], np.float32)
    fc2_b = np.asarray(inputs["fc2_b"], np.float32)

    fwp0, fwp1, fhc_m, fhs_m, fhsm_m, iwrt_m, iwit_m = _dft_consts()
    xf = x.reshape(TOK, C)
    fc1m_m = (g2[:, None] * fc1_w).astype(np.float32)          # (768, 3072)
    uneg_m = (-fc1m_m.sum(0, dtype=np.float64)).astype(np.float32)[None, :]
    gbias_v = (fc1_b + be2 @ fc1_w).astype(np.float32)         # (3072,)
    gbias_m = np.ascontiguousarray(gbias_v.reshape(NMO, 128).T)  # (128, 24)
    fc2b_m = np.ascontiguousarray(fc2_b.reshape(NCC, 128).T)
    g1f_m = np.ascontiguousarray(g1.reshape(NCC, 128).T)
    be1f_m = np.ascontiguousarray(be1.reshape(NCC, 128).T)
    ones1 = np.ones((1, 128), np.float32)
    ones128 = np.ones((128, 1), np.float32)
    onesrow = np.ones((1, 2 * NM), np.float32)
    ident = np.eye(128, dtype=np.float32)

    in_maps = []
    for k in range(NCORES):
        ck = slice(k * BS, (k + 1) * BS)
        xw_k = np.ascontiguousarray(x[:, :, :, ck].transpose(2, 0, 1, 3))
        xc_k = np.ascontiguousarray(xf[k * TSH:(k + 1) * TSH, :].T)
        w1r_k = np.ascontiguousarray(w1[k, :, :, 0])
        w1i_k = np.ascontiguousarray(w1[k, :, :, 1])
        w2r_k = np.ascontiguousarray(w2[k, :, :, 0])
        w2i_k = np.ascontiguousarray(w2[k, :, :, 1])
        b2r_k = b2[k, :, 0]; b2i_k = b2[k, :, 1]
        zr = np.zeros((1, BS), np.float32)
        mask = np.zeros((NCORES, 128), np.float32); mask[k, :] = 1.0
        in_maps.append({
            "xw": xw_k, "xc": xc_k,
            "fwp0": fwp0, "fwp1": fwp1, "fhc": fhc_m, "fhs": fhs_m,
            "fhsm": fhsm_m, "iwrt": iwrt_m, "iwit": iwit_m,
            "w1r": w1r_k, "w1i": w1i_k, "w1im": -w1i_k,
            "b1r": b1[k, :, 0:1].copy(), "b1i": b1[k, :, 1:2].copy(),
            "w2a": np.concatenate([w2r_k, b2r_k[None, :]], 0),
            "w2b": np.concatenate([-w2i_k, zr], 0),
            "w2c": np.concatenate([w2r_k, zr], 0),
            "w2d": np.concatenate([w2i_k, b2i_k[None, :]], 0),
            "g1col": g1[ck][:, None].copy(),
            "b2rr": b2r_k[None, :].copy(), "b2ir": b2i_k[None, :].copy(),
            "spike": (be1[ck] * SQN)[:, None].astype(np.float32),
            "fc1m": fc1m_m, "uneg": uneg_m, "gbias": gbias_m,
            "fc2w": fc2_w, "fc2b": fc2b_m, "g1f": g1f_m, "be1f": be1f_m,
            "ones1": ones1, "ones128": ones128, "onesrow": onesrow,
            "ident": ident, "mask128": mask,
        })
    return in_maps


def kernel(**inputs):
    nc = _build_nc()
    in_maps = _host_prep(inputs)
    res = run_bass_kernel_spmd(nc, in_maps, core_ids=list(range(NCORES)))
    outs = [res.results[j]["out"] for j in range(NCORES)]
    full = np.concatenate(outs, axis=0).reshape(B, H, W, C)
    return full.astype(np.float32)



# revision 21
# speedup vs baseline: 6905.1613x; 6905.1613x over previous
"""AFNO transformer block on 8 Trainium2 NeuronCores (bf16).

Distribution:
  Phase 1 (channel-block sharded): core k owns channels [96k, 96k+96).
    z loaded once in bf16 as [90w, 96c, 90h] tiles per (b, wc-half); LN1
    partial stats (reduce over c) -> per-batch AllReduce (token-major
    [2, 16200]) -> LN1 applied in place -> spectral path: F1 (W-DFT,
    flip), F2 (H-DFT, flip, r/i packed into one PSUM), block complex MLP
    (layer1 weight-stationary, layer2 flip packed), inverse H-DFT (flip,
    packed), inverse W-DFT (weight-stationary over wf).
  Two AllToAlls (one per batch, bf16), overlapped: a2a_0 runs during
    b=1's spectral chain, a2a_1 during phase-2 b=0 tiles.
  Phase 2 (token sharded): core j owns tokens [2025j, 2025(j+1)) of each
    batch. h = filt + LN1(x) + x assembled in bf16, LN2 folded into fc1
    (uneg rank-1 matmul + r2 broadcast), fc1 -> Gelu -> fc2 -> residual
    -> strided DMA straight to token-major output.
"""
import math
import numpy as np
import ml_dtypes

import concourse.bass as bass
import concourse.mybir as mybir
import concourse.tile as tile
from concourse import bacc
from concourse.bass_utils import run_bass_kernel_spmd

F32 = mybir.dt.float32
BF16 = mybir.dt.bfloat16
AF = mybir.ActivationFunctionType
OP = mybir.AluOpType
AX = mybir.AxisListType

NCORES = 8
B, H, W, C = 2, 90, 180, 768
BS = 96            # channels per core / AFNO block size
KW = 46            # kept W-frequency modes
HID = 3072
LAM = 0.01
EPS = 1e-5
TOKB = H * W       # 16200 tokens per batch
TSB = TOKB // NCORES   # 2025 tokens per (core, batch)
TSH = 2 * TSB      # 4050 tokens per core
NM = KW * H        # 4140 modes per block
SQN = math.sqrt(H * W)
NCC = 6            # 768/128
NMO = 24           # 3072/128
TT = 405           # phase-2 token tile width
NT = TSB // TT     # 5 tiles per batch
M1CH = 460         # MLP1 chunk (4140 = 9*460)
BF = ml_dtypes.bfloat16


def _dft_consts():
    wv = np.arange(W, dtype=np.float64)[:, None]
    wf = np.arange(KW, dtype=np.float64)[None, :]
    ang = 2.0 * np.pi * wv * wf / W
    fwr = np.cos(ang) / math.sqrt(W)
    fwi = -np.sin(ang) / math.sqrt(W)
    fwpack = np.concatenate([fwr, fwi], axis=1)          # (180, 92)
    hv = np.arange(H, dtype=np.float64)[:, None]
    hf = np.arange(H, dtype=np.float64)[None, :]
    angh = 2.0 * np.pi * hv * hf / H
    fhc = np.cos(angh) / math.sqrt(H)
    fhs = np.sin(angh) / math.sqrt(H)
    fhsm = -fhs
    alpha = np.ones(KW); alpha[1:] = 2.0
    iwr = alpha[None, :] * np.cos(ang) / math.sqrt(W)    # (180, 46)
    iwi = -alpha[None, :] * np.sin(ang) / math.sqrt(W)
    iwrt = np.ascontiguousarray(iwr.T)                   # (46, 180)
    iwit = np.ascontiguousarray(iwi.T)
    c = {}
    c["fwp0"] = fwpack[:90]
    c["fwp1"] = fwpack[90:]
    c["f2a"] = np.concatenate([fhc, fhsm], axis=1)       # (90, 180)
    c["f2b"] = np.concatenate([fhs, fhc], axis=1)
    c["iha"] = np.concatenate([fhc, fhs], axis=1)
    c["ihb"] = np.concatenate([fhsm, fhc], axis=1)
    c["iwrt"] = iwrt
    c["iwit"] = iwit
    return {k: np.ascontiguousarray(v).astype(BF) for k, v in c.items()}


def _send_pieces(j):
    """(h0,h1,w0,w1) global-w pieces covering dest j's tokens of a batch."""
    s0, e0 = TSB * j, TSB * (j + 1)
    pieces, t = [], s0
    while t < e0:
        h = t // W
        w0 = t - h * W
        if w0 != 0 or e0 - t < W:
            w1 = min(W, w0 + (e0 - t))
            pieces.append((h, h + 1, w0, w1))
            t += w1 - w0
        else:
            h1 = min(H, h + (e0 - t) // W)
            pieces.append((h, h1, 0, W))
            t += (h1 - h) * W
    return pieces


def _recv_pieces(cc):
    c0, out, r0 = cc * 128, [], 0
    while r0 < 128:
        s = (c0 + r0) // BS
        ci = (c0 + r0) % BS
        n = min(BS - ci, 128 - r0)
        out.append((r0, s, ci, n))
        r0 += n
    return out


_CACHE = {}


def _build_nc():
    if "nc" in _CACHE:
        return _CACHE["nc"]
    nc = bacc.Bacc("TRN2", target_bir_lowering=False, debug=False,
                   num_devices=NCORES)

    def g(n, s, dt=BF16):
        return nc.dram_tensor(n, s, dt, kind="ExternalInput")

    xw = g("xw", [W, B, BS, H])
    xc = g("xc", [C, TSH])
    fwp0 = g("fwp0", [90, 92]); fwp1 = g("fwp1", [90, 92])
    f2a = g("f2a", [90, 180]); f2b = g("f2b", [90, 180])
    iha = g("iha", [90, 180]); ihb = g("ihb", [90, 180])
    iwrt = g("iwrt", [KW, W]); iwit = g("iwit", [KW, W])
    w1r = g("w1r", [BS, BS]); w1i = g("w1i", [BS, BS]); w1im = g("w1im", [BS, BS])
    b1r = g("b1r", [BS, 1], F32); b1i = g("b1i", [BS, 1], F32)
    b1sr = g("b1sr", [BS, 1], F32); b1si = g("b1si", [BS, 1], F32)
    w2p1 = g("w2p1", [BS, 192]); w2p2 = g("w2p2", [BS, 192])
    b2pk = g("b2pk", [1, 192])
    fc1m = g("fc1m", [C, HID])
    uneg = g("uneg", [1, HID])
    gbias = g("gbias", [128, NMO], F32)
    fc2w = g("fc2w", [HID, C])
    fc2b = g("fc2b", [128, NCC], F32)
    g1f = g("g1f", [128, NCC], F32); be1f = g("be1f", [128, NCC], F32)
    ones1 = g("ones1", [1, 128])
    ones128 = g("ones128", [128, 1])

    out = nc.dram_tensor("out", [C, TSH], F32, kind="ExternalOutput")
    rg = [list(range(NCORES))]

    from contextlib import ExitStack
    with tile.TileContext(nc) as tc:
        with ExitStack() as st0:
            cp = st0.enter_context(tc.tile_pool(name="const", bufs=1))
            dram = st0.enter_context(tc.tile_pool(name="dram", bufs=1, space="DRAM"))

            def cl(t, shape, dt=BF16):
                nm = f"c_{t.name}"
                s = cp.tile(shape, dt, name=nm, tag=nm)
                nc.sync.dma_start(s[:], t[:])
                return s

            c_fwp0 = cl(fwp0, [90, 92]); c_fwp1 = cl(fwp1, [90, 92])
            c_f2a = cl(f2a, [90, 180]); c_f2b = cl(f2b, [90, 180])
            c_iha = cl(iha, [90, 180]); c_ihb = cl(ihb, [90, 180])
            c_iwrt = cl(iwrt, [KW, W]); c_iwit = cl(iwit, [KW, W])
            c_w1r = cl(w1r, [BS, BS]); c_w1i = cl(w1i, [BS, BS])
            c_w1im = cl(w1im, [BS, BS])
            c_b1r = cl(b1r, [BS, 1], F32); c_b1i = cl(b1i, [BS, 1], F32)
            c_b1sr = cl(b1sr, [BS, 1], F32); c_b1si = cl(b1si, [BS, 1], F32)
            c_w2p1 = cl(w2p1, [BS, 192]); c_w2p2 = cl(w2p2, [BS, 192])
            c_b2pk = cl(b2pk, [1, 192])
            c_gbias = cl(gbias, [128, NMO], F32)
            c_fc2b = cl(fc2b, [128, NCC], F32)
            c_g1f = cl(g1f, [128, NCC], F32); c_be1f = cl(be1f, [128, NCC], F32)
            c_uneg = cl(uneg, [1, HID])
            c_ones1 = cl(ones1, [1, 128]); c_ones128 = cl(ones128, [128, 1])
            c_eps = cp.tile([128, 1], F32, name="c_eps")
            nc.vector.memset(c_eps[:], EPS)

            st_in = [dram.tile([2, TOKB], F32, name=f"st_in{b_}") for b_ in range(B)]
            st_out = [dram.tile([2, TOKB], F32, name=f"st_out{b_}") for b_ in range(B)]
            a2a_in = [dram.tile([NCORES, BS, TSB], BF16, name=f"a2a_in{b_}")
                      for b_ in range(B)]
            a2a_out = [dram.tile([NCORES, BS, TSB], BF16, name=f"a2a_out{b_}")
                       for b_ in range(B)]

            # ================= phase 1 =================
            with ExitStack() as st1:
                zp = st1.enter_context(tc.tile_pool(name="zp", bufs=2))
                sqp = st1.enter_context(tc.tile_pool(name="sqp", bufs=1))
                clp = st1.enter_context(tc.tile_pool(name="clp", bufs=2))
                stp = st1.enter_context(tc.tile_pool(name="stats", bufs=1))
                ybo2 = st1.enter_context(tc.tile_pool(name="ybo2", bufs=2))
                zbp = st1.enter_context(tc.tile_pool(name="zbp", bufs=1))
                o1p = st1.enter_context(tc.tile_pool(name="o1p", bufs=1))
                u2p = st1.enter_context(tc.tile_pool(name="u2p", bufs=1))
                s2p = st1.enter_context(tc.tile_pool(name="s2p", bufs=1))
                pp = st1.enter_context(tc.tile_pool(name="psum1", bufs=8,
                                                    space="PSUM"))
                zhs = {}

                def stk(t, kind):
                    return bass.AP(tensor=t[:].tensor,
                                   offset=t[:].offset + kind * TOKB,
                                   ap=[[90, 90], [8100, 2], [1, 90]])

                def emit_loads_stats(b):
                    """Load z (bf16), partial LN1 stats, AllReduce trigger."""
                    eng = nc.vector
                    zh = []
                    for wc in range(2):
                        zt = zp.tile([90, BS, H], BF16, tag="zh",
                                     name=f"zh{b}{wc}")
                        nc.scalar.dma_start(
                            zt[:], xw[wc * 90:(wc + 1) * 90, b, :, :])
                        zh.append(zt)
                    zhs[b] = zh
                    s_sum = stp.tile([90, 2, H], F32, tag="ssum")
                    s_sq = stp.tile([90, 2, H], F32, tag="ssq")
                    s_t = stp.tile([90, H], F32, tag="st_t")
                    zhs[b, "sum"] = s_sum
                    zhs[b, "sq"] = s_sq
                    for wc in range(2):
                        zt = zh[wc]
                        base = zt[:]
                        zv = bass.AP(tensor=base.tensor, offset=base.offset,
                                     ap=[list(base.ap[0]), [1, H], [H, BS]])
                        eng.reduce_sum(s_sum[:, wc, :], zv, axis=AX.X)
                        # squared sums in 24-channel blocks (small scratch)
                        for blk in range(4):
                            sqt = sqp.tile([90, 24, H], BF16, tag="sqt")
                            nc.scalar.activation(
                                out=sqt[:], in_=zt[:, blk * 24:(blk + 1) * 24, :],
                                func=AF.Square)
                            sb = sqt[:]
                            sv = bass.AP(tensor=sb.tensor, offset=sb.offset,
                                         ap=[list(sb.ap[0]), [1, H], [H, 24]])
                            if blk == 0:
                                eng.reduce_sum(s_sq[:, wc, :], sv, axis=AX.X)
                            else:
                                eng.reduce_sum(s_t[:], sv, axis=AX.X)
                                eng.tensor_add(s_sq[:, wc, :], s_sq[:, wc, :],
                                               s_t[:])
                    nc.sync.dma_start(stk(st_in[b], 0), s_sum[:])
                    nc.sync.dma_start(stk(st_in[b], 1), s_sq[:])
                    nc.gpsimd.collective_compute(
                        "AllReduce", OP.add, replica_groups=rg,
                        ins=[st_in[b][:].opt()], outs=[st_out[b][:].opt()])

                def emit_post_stats(b):
                    """st recv, m/r, phase-2 rows, LN1 apply in place."""
                    s_sum, s_sq = zhs[b, "sum"], zhs[b, "sq"]
                    nc.sync.dma_start(s_sum[:], stk(st_out[b], 0))
                    nc.sync.dma_start(s_sq[:], stk(st_out[b], 1))
                    s_m = stp.tile([90, 2, H], F32, tag="sm")
                    s_r = stp.tile([90, 2, H], F32, tag="sr")
                    s_v = stp.tile([90, 2, H], F32, tag="sv")
                    nc.vector.tensor_scalar(out=s_m[:], in0=s_sum[:],
                                            scalar1=1.0 / C, scalar2=None,
                                            op0=OP.mult)
                    nc.vector.tensor_scalar(out=s_r[:], in0=s_sq[:],
                                            scalar1=1.0 / C, scalar2=None,
                                            op0=OP.mult)
                    nc.vector.tensor_mul(s_v[:], s_m[:], s_m[:])
                    nc.vector.tensor_sub(s_r[:], s_r[:], s_v[:])
                    nc.scalar.activation(out=s_r[:], in_=s_r[:],
                                         func=AF.Sqrt, bias=c_eps[:90])
                    nc.vector.reciprocal(s_r[:], s_r[:])
                    s_rb = stp.tile([90, 2, H], BF16, tag="srb")
                    s_mrb = stp.tile([90, 2, H], BF16, tag="smrb")
                    nc.vector.tensor_copy(s_rb[:], s_r[:])
                    nc.vector.tensor_mul(s_v[:], s_m[:], s_r[:])
                    nc.vector.tensor_copy(s_mrb[:], s_v[:])

                    for wc in range(2):
                        zt = zhs[b][wc]

                        def bc(t):
                            a = t[:, wc, :]
                            return bass.AP(tensor=a.tensor, offset=a.offset,
                                           ap=[list(a.ap[0]), [0, BS], [1, H]])
                        nc.vector.tensor_mul(zt[:], zt[:], bc(s_rb))
                        nc.vector.tensor_sub(zt[:], zt[:], bc(s_mrb))

                def emit_f1(b):
                    zh = zhs[b]
                    yb = ybo2.tile([90, BS, 92], BF16, tag="ybo2", name=f"yb{b}")
                    zhs[b, "yb"] = yb
                    for gi, c0 in enumerate(range(0, BS, 4)):
                        pf = pp.tile([90, 4 * 92], F32, tag="pp", name="psf1")
                        for ci in range(4):
                            c = c0 + ci
                            nc.tensor.matmul(pf[:, ci * 92:(ci + 1) * 92],
                                             zh[0][:, c, :], c_fwp0[:],
                                             start=True, stop=False)
                            nc.tensor.matmul(pf[:, ci * 92:(ci + 1) * 92],
                                             zh[1][:, c, :], c_fwp1[:],
                                             start=False, stop=True)
                        dst = yb[:, c0:c0 + 4, :]
                        src = pf[:].rearrange("p (a b) -> p a b", a=4)
                        if gi % 2 == 0:
                            nc.scalar.activation(out=dst, in_=src, func=AF.Copy)
                        else:
                            nc.vector.tensor_copy(dst, src)

                def emit_f2(b):
                    yb = zhs[b, "yb"]
                    # ---- F2 (flip, packed r/i)
                    zb = zbp.tile([BS, 2, KW, H], BF16, tag="zb", name=f"zb{b}")
                    zhs[b, "zb"] = zb
                    for wf in range(KW):
                        pz = pp.tile([BS, 180], F32, tag="pp", name="psf2")
                        nc.tensor.matmul(pz[:], yb[:, :, wf], c_f2a[:],
                                         start=True, stop=False)
                        nc.tensor.matmul(pz[:], yb[:, :, 46 + wf], c_f2b[:],
                                         start=False, stop=True)
                        nc.vector.tensor_copy(
                            zb[:, :, wf, :],
                            pz[:].rearrange("p (a b) -> p a b", a=2))

                def emit_rest(b):
                    zb = zhs[b, "zb"]
                    # ---- block MLP layer 1 (weight-stationary) + Relu
                    o1 = o1p.tile([BS, 2, NM], BF16, tag="o1", name=f"o1{b}")
                    zr_f = zb[:, 0].rearrange("p a b -> p (a b)")
                    zi_f = zb[:, 1].rearrange("p a b -> p (a b)")
                    for ch in range(9):
                        n0 = ch * M1CH
                        zr_s = zr_f[:, n0:n0 + M1CH]
                        zi_s = zi_f[:, n0:n0 + M1CH]
                        por = pp.tile([BS, M1CH], F32, tag="pp", name="pso1r")
                        nc.tensor.matmul(por[:], c_w1r[:], zr_s,
                                         start=True, stop=False)
                        nc.tensor.matmul(por[:], c_w1im[:], zi_s,
                                         start=False, stop=True)
                        poi = pp.tile([BS, M1CH], F32, tag="pp", name="pso1i")
                        nc.tensor.matmul(poi[:], c_w1i[:], zr_s,
                                         start=True, stop=False)
                        nc.tensor.matmul(poi[:], c_w1r[:], zi_s,
                                         start=False, stop=True)
                        if ch == 0:
                            # be1 spike contribution on mode (0,0) only
                            nc.vector.tensor_scalar(out=por[:, 0:1],
                                                    in0=por[:, 0:1],
                                                    scalar1=c_b1sr[:],
                                                    scalar2=None, op0=OP.add)
                            nc.vector.tensor_scalar(out=poi[:, 0:1],
                                                    in0=poi[:, 0:1],
                                                    scalar1=c_b1si[:],
                                                    scalar2=None, op0=OP.add)
                        nc.scalar.activation(out=o1[:, 0, n0:n0 + M1CH],
                                             in_=por[:], func=AF.Relu,
                                             bias=c_b1r[:])
                        nc.scalar.activation(out=o1[:, 1, n0:n0 + M1CH],
                                             in_=poi[:], func=AF.Relu,
                                             bias=c_b1i[:])

                    # ---- block MLP layer 2 (flip, packed) + softshrink
                    o2 = ybo2.tile([H, 2, KW, BS], BF16, tag="ybo2",
                                   name=f"o2{b}")
                    for wf in range(KW):
                        lr = o1[:, 0, wf * H:(wf + 1) * H]
                        li = o1[:, 1, wf * H:(wf + 1) * H]
                        pm = pp.tile([H, 192], F32, tag="pp", name="pso2")
                        nc.tensor.matmul(pm[:], lr, c_w2p1[:],
                                         start=True, stop=False)
                        nc.tensor.matmul(pm[:], li, c_w2p2[:],
                                         start=False, stop=False)
                        nc.tensor.matmul(pm[:], c_ones1[:, 0:H], c_b2pk[:],
                                         start=False, stop=True)
                        clip = clp.tile([H, 192], F32, tag="clip")
                        nc.vector.tensor_scalar(out=clip[:], in0=pm[:],
                                                scalar1=-LAM, scalar2=LAM,
                                                op0=OP.max, op1=OP.min)
                        nc.vector.tensor_tensor(
                            out=o2[:, :, wf, :],
                            in0=pm[:].rearrange("p (a b) -> p a b", a=2),
                            in1=clip[:].rearrange("p (a b) -> p a b", a=2),
                            op=OP.subtract)

                    # ---- inverse H-DFT (flip, packed) -> u2 [46, 2, 96, 90]
                    u2 = u2p.tile([KW, 2, BS, H], BF16, tag="u2", name=f"u2{b}")
                    for c in range(BS):
                        lr = o2[:, 0, :, c]
                        li = o2[:, 1, :, c]
                        pu = pp.tile([KW, 180], F32, tag="pp", name="psu")
                        nc.tensor.matmul(pu[:], lr, c_iha[:],
                                         start=True, stop=False)
                        nc.tensor.matmul(pu[:], li, c_ihb[:],
                                         start=False, stop=True)
                        dst = u2[:, :, c, :]
                        src = pu[:].rearrange("p (a b) -> p a b", a=2)
                        if c % 2 == 0:
                            nc.scalar.activation(out=dst, in_=src, func=AF.Copy)
                        else:
                            nc.vector.tensor_copy(dst, src)

                    # ---- inverse W-DFT (flip) -> s2 [90h, 96c, 180w]
                    s2 = s2p.tile([H, BS, W], BF16, tag="s2", name=f"s2{b}")
                    for c in range(BS):
                        pf = pp.tile([H, W], F32, tag="pp", name="psw")
                        nc.tensor.matmul(pf[:], u2[:, 0, c, :], c_iwrt[:],
                                         start=True, stop=False)
                        nc.tensor.matmul(pf[:], u2[:, 1, c, :], c_iwit[:],
                                         start=False, stop=True)
                        if c % 2 == 0:
                            nc.scalar.activation(out=s2[:, c, :], in_=pf[:],
                                                 func=AF.Copy)
                        else:
                            nc.vector.tensor_copy(s2[:, c, :], pf[:])

                    # ---- a2a send pieces (SBUF -> DRAM, w-contiguous)
                    for j in range(NCORES):
                        t0 = TSB * j
                        for (h0, h1, w0, w1) in _send_pieces(j):
                            src = s2[h0:h1, :, w0:w1]
                            dst = bass.AP(
                                tensor=a2a_in[b][:].tensor,
                                offset=(a2a_in[b][:].offset
                                        + j * BS * TSB
                                        + (h0 * W + w0 - t0)),
                                ap=[[W, h1 - h0], [TSB, BS], [1, w1 - w0]])
                            nc.sync.dma_start(dst, src)

                # emission order chosen so collective triggers never block
                # earlier-needed work on the same engine queue
                emit_loads_stats(0)
                emit_post_stats(0)
                emit_f1(0)
                emit_f2(0)
                emit_loads_stats(1)     # AR1 triggers before a2a_0
                emit_rest(0)
                nc.gpsimd.collective_compute(
                    "AllToAll", OP.bypass, replica_groups=rg,
                    ins=[a2a_in[0][:].opt()], outs=[a2a_out[0][:].opt()])
                emit_post_stats(1)
                emit_f1(1)
                emit_f2(1)
                emit_rest(1)

            # ================= phase 2 =================
            with ExitStack() as st2:
                fc1p = st2.enter_context(tc.tile_pool(name="fc1p", bufs=1))
                fc2p = st2.enter_context(tc.tile_pool(name="fc2p", bufs=1))
                xtp = st2.enter_context(tc.tile_pool(name="xtp", bufs=12))
                hrp = st2.enter_context(tc.tile_pool(name="hrp", bufs=12))
                hbp = st2.enter_context(tc.tile_pool(name="hbp", bufs=12))
                hip = st2.enter_context(tc.tile_pool(name="hip", bufs=1))
                t1p = st2.enter_context(tc.tile_pool(name="t1p", bufs=4))
                rbp = st2.enter_context(tc.tile_pool(name="rbp", bufs=6))
                rw2 = st2.enter_context(tc.tile_pool(name="rw2", bufs=1))
                outp = st2.enter_context(tc.tile_pool(name="outp", bufs=2))
                ph = st2.enter_context(tc.tile_pool(name="ph", bufs=2, space="PSUM"))
                po = st2.enter_context(tc.tile_pool(name="po", bufs=2, space="PSUM"))
                pst = st2.enter_context(tc.tile_pool(name="pst", bufs=2, space="PSUM"))
                pbc = st2.enter_context(tc.tile_pool(name="pbc", bufs=2, space="PSUM"))

                c_fc1 = [fc1p.tile([128, HID], BF16, tag=f"fc1_{i}", name=f"cfc1_{i}")
                         for i in range(NCC)]
                for i in range(NCC):
                    nc.gpsimd.dma_start(c_fc1[i][:], fc1m[i * 128:(i + 1) * 128, :])
                c_fc2 = [fc2p.tile([128, C], BF16, tag=f"fc2_{i}", name=f"cfc2_{i}")
                         for i in range(NMO)]
                for i in range(NMO):
                    nc.gpsimd.dma_start(c_fc2[i][:], fc2w[i * 128:(i + 1) * 128, :])

                for tb in range(B):
                    for tt in range(NT):
                        t0 = tt * TT
                        tg = tb * TSB + t0
                        # xc slices + local LN1 stats (all channels on hand)
                        xcts = []
                        ps_m = pst.tile([1, TT], F32, tag="pst")
                        ps_qx = pst.tile([1, TT], F32, tag="pst")
                        for cc in range(NCC):
                            xct = xtp.tile([128, TT], BF16, tag="xct")
                            nc.gpsimd.dma_start(
                                xct[:], xc[cc * 128:(cc + 1) * 128, tg:tg + TT])
                            xcts.append(xct)
                        for cc in range(NCC):
                            nc.tensor.matmul(ps_m[:], c_ones128[:], xcts[cc][:],
                                             start=(cc == 0),
                                             stop=(cc == NCC - 1))
                        for cc in range(NCC):
                            xsq = t1p.tile([128, TT], BF16, tag="hsq")
                            nc.vector.tensor_mul(xsq[:], xcts[cc][:],
                                                 xcts[cc][:])
                            nc.tensor.matmul(ps_qx[:], c_ones128[:], xsq[:],
                                             start=(cc == 0),
                                             stop=(cc == NCC - 1))
                        m1r = rw2.tile([1, TT], F32, tag="m1r")
                        r1r = rw2.tile([1, TT], F32, tag="r1r")
                        v1 = rw2.tile([1, TT], F32, tag="v1")
                        nc.vector.tensor_scalar(out=m1r[:], in0=ps_m[:],
                                                scalar1=1.0 / C, scalar2=None,
                                                op0=OP.mult)
                        nc.vector.tensor_scalar(out=r1r[:], in0=ps_qx[:],
                                                scalar1=1.0 / C, scalar2=None,
                                                op0=OP.mult)
                        nc.vector.tensor_mul(v1[:], m1r[:], m1r[:])
                        nc.vector.tensor_sub(r1r[:], r1r[:], v1[:])
                        nc.scalar.activation(out=r1r[:], in_=r1r[:],
                                             func=AF.Sqrt, bias=c_eps[:1])
                        nc.vector.reciprocal(r1r[:], r1r[:])
                        r1bf = rw2.tile([1, TT], BF16, tag="r1bf")
                        mr1bf = rw2.tile([1, TT], BF16, tag="mr1bf")
                        nc.vector.tensor_copy(r1bf[:], r1r[:])
                        nc.vector.tensor_mul(m1r[:], m1r[:], r1r[:])
                        nc.vector.tensor_copy(mr1bf[:], m1r[:])
                        r1b = rbp.tile([128, TT], F32, tag="r1b")
                        mr1b = rbp.tile([128, TT], F32, tag="mr1b")
                        for rows, bt in ((r1bf, r1b), (mr1bf, mr1b)):
                            pb = pbc.tile([128, TT], F32, tag="pbc")
                            nc.tensor.matmul(pb[:], c_ones1[:], rows[:],
                                             start=True, stop=True)
                            nc.vector.tensor_copy(bt[:], pb[:])

                        htbs = []
                        for cc in range(NCC):
                            xct = xcts[cc]
                            htr = hrp.tile([128, TT], BF16, tag="htr")
                            for (r0, sc, ci, n) in _recv_pieces(cc):
                                nc.gpsimd.dma_start(
                                    htr[r0:r0 + n, :],
                                    a2a_out[tb][sc, ci:ci + n, t0:t0 + TT])
                            t1 = t1p.tile([128, TT], F32, tag="t1")
                            nc.vector.tensor_mul(t1[:], xct[:], r1b[:])
                            nc.vector.tensor_sub(t1[:], t1[:], mr1b[:])
                            nc.vector.tensor_scalar(out=t1[:], in0=t1[:],
                                                    scalar1=c_g1f[:, cc:cc + 1],
                                                    scalar2=c_be1f[:, cc:cc + 1],
                                                    op0=OP.mult, op1=OP.add)
                            htb = hbp.tile([128, TT], BF16, tag="htb")
                            nc.vector.tensor_add(htb[:], htr[:], xct[:])
                            nc.vector.tensor_add(htb[:], htb[:], t1[:])
                            htbs.append(htb)

                        # LN2 stats via ones-matmuls
                        ps_s = pst.tile([1, TT], F32, tag="pst")
                        ps_q = pst.tile([1, TT], F32, tag="pst")
                        for cc in range(NCC):
                            nc.tensor.matmul(ps_s[:], c_ones128[:], htbs[cc][:],
                                             start=(cc == 0),
                                             stop=(cc == NCC - 1))
                        for cc in range(NCC):
                            hsq = t1p.tile([128, TT], BF16, tag="hsq")
                            nc.vector.tensor_mul(hsq[:], htbs[cc][:],
                                                 htbs[cc][:])
                            nc.tensor.matmul(ps_q[:], c_ones128[:], hsq[:],
                                             start=(cc == 0),
                                             stop=(cc == NCC - 1))
                        m2r = rw2.tile([1, TT], F32, tag="m2r")
                        r2r = rw2.tile([1, TT], F32, tag="r2r")
                        vv = rw2.tile([1, TT], F32, tag="vv")
                        nc.vector.tensor_scalar(out=m2r[:], in0=ps_s[:],
                                                scalar1=1.0 / C, scalar2=None,
                                                op0=OP.mult)
                        nc.vector.tensor_scalar(out=r2r[:], in0=ps_q[:],
                                                scalar1=1.0 / C, scalar2=None,
                                                op0=OP.mult)
                        nc.vector.tensor_mul(vv[:], m2r[:], m2r[:])
                        nc.vector.tensor_sub(r2r[:], r2r[:], vv[:])
                        nc.scalar.activation(out=r2r[:], in_=r2r[:],
                                             func=AF.Sqrt, bias=c_eps[:1])
                        nc.vector.reciprocal(r2r[:], r2r[:])
                        m2b = rw2.tile([1, TT], BF16, tag="m2b")
                        r2bf = rw2.tile([1, TT], BF16, tag="r2bf")
                        nc.vector.tensor_copy(m2b[:], m2r[:])
                        nc.vector.tensor_copy(r2bf[:], r2r[:])
                        r2s = rbp.tile([128, TT], F32, tag="r2s")
                        pb2 = pbc.tile([128, TT], F32, tag="pbc")
                        nc.tensor.matmul(pb2[:], c_ones1[:], r2bf[:],
                                         start=True, stop=True)
                        nc.vector.tensor_copy(r2s[:], pb2[:])

                        # fc1 + LN2 fold + gelu -> hid (bf16)
                        hid = hip.tile([128, NMO, TT], BF16, tag="hid")
                        for mo in range(NMO):
                            php = ph.tile([128, TT], F32, tag="ph")
                            for cc in range(NCC):
                                nc.tensor.matmul(
                                    php[:],
                                    c_fc1[cc][:, mo * 128:(mo + 1) * 128],
                                    htbs[cc][:],
                                    start=(cc == 0), stop=False)
                            nc.tensor.matmul(php[:],
                                             c_uneg[:, mo * 128:(mo + 1) * 128],
                                             m2b[:], start=False, stop=True)
                            t2 = t1p.tile([128, TT], F32, tag="t2")
                            nc.vector.tensor_mul(t2[:], php[:], r2s[:])
                            nc.scalar.activation(out=hid[:, mo, :], in_=t2[:],
                                                 func=AF.Gelu,
                                                 bias=c_gbias[:, mo:mo + 1])

                        # fc2 + bias + residual -> channel-major out (host
                        # transposes)
                        for co in range(NCC):
                            pop = po.tile([128, TT], F32, tag="po")
                            for ho in range(NMO):
                                nc.tensor.matmul(
                                    pop[:],
                                    c_fc2[ho][:, co * 128:(co + 1) * 128],
                                    hid[:, ho, :],
                                    start=(ho == 0), stop=(ho == NMO - 1))
                            of = outp.tile([128, TT], F32, tag="of")
                            nc.vector.scalar_tensor_tensor(
                                out=of[:], in0=pop[:],
                                scalar=c_fc2b[:, co:co + 1], in1=htbs[co][:],
                                op0=OP.add, op1=OP.add)
                            nc.sync.dma_start(
                                out[co * 128:(co + 1) * 128, tg:tg + TT],
                                of[:])

                    if tb == 0:
                        # emitted after tb=0 recv DMAs so its trigger wait
                        # (b=1 sends) never delays them on the gpsimd queue
                        nc.gpsimd.collective_compute(
                            "AllToAll", OP.bypass, replica_groups=rg,
                            ins=[a2a_in[1][:].opt()],
                            outs=[a2a_out[1][:].opt()])

    nc.compile()
    _CACHE["nc"] = nc
    return nc


def _host_prep(inputs):
    x = np.ascontiguousarray(np.asarray(inputs["x"], dtype=np.float32))
    g1 = np.asarray(inputs["g1"], np.float32); be1 = np.asarray(inputs["be1"], np.float32)
    g2 = np.asarray(inputs["g2"], np.float32); be2 = np.asarray(inputs["be2"], np.float32)
    w1 = np.asarray(inputs["w1"], np.float32); b1 = np.asarray(inputs["b1"], np.float32)
    w2 = np.asarray(inputs["w2"], np.float32); b2 = np.asarray(inputs["b2"], np.float32)
    fc1_w = np.asarray(inputs["fc1_w"], np.float32)
    fc1_b = np.asarray(inputs["fc1_b"], np.float32)
    fc2_w = np.asarray(inputs["fc2_w"], np.float32)
    fc2_b = np.asarray(inputs["fc2_b"], np.float32)

    dft = _dft_consts()
    xf = x.reshape(B, TOKB, C)
    fc1s = g2[:, None] * fc1_w
    fc1m_m = fc1s.astype(BF)                                     # (768, 3072)
    uneg_m = (-fc1s.sum(0, dtype=np.float64)).astype(BF)[None, :]
    gbias_v = (fc1_b + be2 @ fc1_w).astype(np.float32)           # (3072,)
    gbias_m = np.ascontiguousarray(gbias_v.reshape(NMO, 128).T)  # (128, 24)
    fc2b_m = np.ascontiguousarray(fc2_b.reshape(NCC, 128).T)
    g1f_m = np.ascontiguousarray(g1.reshape(NCC, 128).T)
    be1f_m = np.ascontiguousarray(be1.reshape(NCC, 128).T)
    ones1 = np.ones((1, 128), BF)
    ones128 = np.ones((128, 1), BF)

    in_maps = []
    for k in range(NCORES):
        ck = slice(k * BS, (k + 1) * BS)
        xw_k = np.ascontiguousarray(
            x[:, :, :, ck].transpose(2, 0, 3, 1)).astype(BF)     # [W,B,BS,H]
        xc_k = np.ascontiguousarray(
            np.concatenate([xf[0, k * TSB:(k + 1) * TSB],
                            xf[1, k * TSB:(k + 1) * TSB]], 0).T).astype(BF)
        g1k = g1[ck]
        w1r_k = w1[k, :, :, 0]; w1i_k = w1[k, :, :, 1]
        w1rp = (g1k[:, None] * w1r_k).astype(BF)
        w1ip = (g1k[:, None] * w1i_k).astype(BF)
        w1imp = (-(g1k[:, None] * w1i_k)).astype(BF)
        spike = (be1[ck] * SQN).astype(np.float64)
        b1sr_k = (w1r_k.T.astype(np.float64) @ spike).astype(np.float32)[:, None]
        b1si_k = (w1i_k.T.astype(np.float64) @ spike).astype(np.float32)[:, None]
        w2r_k = w2[k, :, :, 0]; w2i_k = w2[k, :, :, 1]
        w2p1_k = np.concatenate([w2r_k, w2i_k], 1).astype(BF)    # [96, 192]
        w2p2_k = np.concatenate([-w2i_k, w2r_k], 1).astype(BF)
        b2pk_k = np.concatenate([b2[k, :, 0], b2[k, :, 1]])[None, :].astype(BF)
        in_maps.append({
            "xw": xw_k, "xc": xc_k,
            **{n: dft[n] for n in ("fwp0", "fwp1", "f2a", "f2b", "iha", "ihb",
                                   "iwrt", "iwit")},
            "w1r": w1rp, "w1i": w1ip, "w1im": w1imp,
            "b1r": b1[k, :, 0:1].copy(), "b1i": b1[k, :, 1:2].copy(),
            "b1sr": b1sr_k, "b1si": b1si_k,
            "w2p1": w2p1_k, "w2p2": w2p2_k, "b2pk": b2pk_k,
            "fc1m": fc1m_m, "uneg": uneg_m, "gbias": gbias_m,
            "fc2w": fc2_w.astype(BF), "fc2b": fc2b_m,
            "g1f": g1f_m, "be1f": be1f_m,
            "ones1": ones1, "ones128": ones128,
        })
    return in_maps


def kernel(**inputs):
    nc = _build_nc()
    in_maps = _host_prep(inputs)
    res = run_bass_kernel_spmd(nc, in_maps, core_ids=list(range(NCORES)))
    full = np.empty((B, TOKB, C), np.float32)
    for j in range(NCORES):
        o = np.asarray(res.results[j]["out"], np.float32).T   # [4050, 768]
        full[0, j * TSB:(j + 1) * TSB] = o[:TSB]
        full[1, j * TSB:(j + 1) * TSB] = o[TSB:]
    return full.reshape(B, H, W, C)


# revision 26
# speedup vs baseline: 7051.5865x; 1.0212x over previous
"""AFNO transformer block on 8 Trainium2 NeuronCores (bf16).

Distribution:
  Phase 1 (channel-block sharded): core k owns channels [96k, 96k+96).
    z loaded once in bf16 as [90w, 96c, 90h] tiles per (b, wc-half); LN1
    partial stats (reduce over c) -> per-batch AllReduce (token-major
    [2, 16200]) -> LN1 applied in place -> spectral path: F1 (W-DFT,
    flip), F2 (H-DFT, flip, r/i packed into one PSUM), block complex MLP
    (layer1 weight-stationary, layer2 flip packed), inverse H-DFT (flip,
    packed), inverse W-DFT (weight-stationary over wf).
  Two AllToAlls (one per batch, bf16), overlapped: a2a_0 runs during
    b=1's spectral chain, a2a_1 during phase-2 b=0 tiles.
  Phase 2 (token sharded): core j owns tokens [2025j, 2025(j+1)) of each
    batch. h = filt + LN1(x) + x assembled in bf16, LN2 folded into fc1
    (uneg rank-1 matmul + r2 broadcast), fc1 -> Gelu -> fc2 -> residual
    -> strided DMA straight to token-major output.
"""
import math
import numpy as np
import ml_dtypes

import concourse.bass as bass
import concourse.mybir as mybir
import concourse.tile as tile
from concourse import bacc
from concourse.bass_utils import run_bass_kernel_spmd

F32 = mybir.dt.float32
BF16 = mybir.dt.bfloat16
AF = mybir.ActivationFunctionType
OP = mybir.AluOpType
AX = mybir.AxisListType

NCORES = 8
B, H, W, C = 2, 90, 180, 768
BS = 96            # channels per core / AFNO block size
KW = 46            # kept W-frequency modes
HID = 3072
LAM = 0.01
EPS = 1e-5
TOKB = H * W       # 16200 tokens per batch
TSB = TOKB // NCORES   # 2025 tokens per (core, batch)
TSH = 2 * TSB      # 4050 tokens per core
NM = KW * H        # 4140 modes per block
SQN = math.sqrt(H * W)
NCC = 6            # 768/128
NMO = 24           # 3072/128
TT = 405           # phase-2 token tile width
NT = TSB // TT     # 5 tiles per batch
M1CH = 460         # MLP1 chunk (4140 = 9*460)
BF = ml_dtypes.bfloat16


def _dft_consts():
    wv = np.arange(W, dtype=np.float64)[:, None]
    wf = np.arange(KW, dtype=np.float64)[None, :]
    ang = 2.0 * np.pi * wv * wf / W
    fwr = np.cos(ang) / math.sqrt(W)
    fwi = -np.sin(ang) / math.sqrt(W)
    fwpack = np.concatenate([fwr, fwi], axis=1)          # (180, 92)
    hv = np.arange(H, dtype=np.float64)[:, None]
    hf = np.arange(H, dtype=np.float64)[None, :]
    angh = 2.0 * np.pi * hv * hf / H
    fhc = np.cos(angh) / math.sqrt(H)
    fhs = np.sin(angh) / math.sqrt(H)
    fhsm = -fhs
    alpha = np.ones(KW); alpha[1:] = 2.0
    iwr = alpha[None, :] * np.cos(ang) / math.sqrt(W)    # (180, 46)
    iwi = -alpha[None, :] * np.sin(ang) / math.sqrt(W)
    iwrt = np.ascontiguousarray(iwr.T)                   # (46, 180)
    iwit = np.ascontiguousarray(iwi.T)
    c = {}
    c["fwp0"] = fwpack[:90]
    c["fwp1"] = fwpack[90:]
    c["f2a"] = np.concatenate([fhc, fhsm], axis=1)       # (90, 180)
    c["f2b"] = np.concatenate([fhs, fhc], axis=1)
    c["iha"] = np.concatenate([fhc, fhs], axis=1)
    c["ihb"] = np.concatenate([fhsm, fhc], axis=1)
    c["iwrt"] = iwrt
    c["iwit"] = iwit
    return {k: np.ascontiguousarray(v).astype(BF) for k, v in c.items()}


def _send_pieces(j):
    """(h0,h1,w0,w1) global-w pieces covering dest j's tokens of a batch."""
    s0, e0 = TSB * j, TSB * (j + 1)
    pieces, t = [], s0
    while t < e0:
        h = t // W
        w0 = t - h * W
        if w0 != 0 or e0 - t < W:
            w1 = min(W, w0 + (e0 - t))
            pieces.append((h, h + 1, w0, w1))
            t += w1 - w0
        else:
            h1 = min(H, h + (e0 - t) // W)
            pieces.append((h, h1, 0, W))
            t += (h1 - h) * W
    return pieces


def _recv_pieces(cc):
    c0, out, r0 = cc * 128, [], 0
    while r0 < 128:
        s = (c0 + r0) // BS
        ci = (c0 + r0) % BS
        n = min(BS - ci, 128 - r0)
        out.append((r0, s, ci, n))
        r0 += n
    return out


_CACHE = {}


def _build_nc():
    if "nc" in _CACHE:
        return _CACHE["nc"]
    nc = bacc.Bacc("TRN2", target_bir_lowering=False, debug=False,
                   num_devices=NCORES)

    def g(n, s, dt=BF16):
        return nc.dram_tensor(n, s, dt, kind="ExternalInput")

    xw = g("xw", [W, B, H, BS])
    xc = g("xc", [C, TSH])
    fwp0 = g("fwp0", [90, 92]); fwp1 = g("fwp1", [90, 92])
    f2a = g("f2a", [90, 180]); f2b = g("f2b", [90, 180])
    iha = g("iha", [90, 180]); ihb = g("ihb", [90, 180])
    iwrt = g("iwrt", [KW, W]); iwit = g("iwit", [KW, W])
    w1r = g("w1r", [BS, BS]); w1i = g("w1i", [BS, BS]); w1im = g("w1im", [BS, BS])
    b1r = g("b1r", [BS, 1], F32); b1i = g("b1i", [BS, 1], F32)
    b1sr = g("b1sr", [BS, 1], F32); b1si = g("b1si", [BS, 1], F32)
    w2p1 = g("w2p1", [BS, 192]); w2p2 = g("w2p2", [BS, 192])
    b2pk = g("b2pk", [1, 192])
    fc1m = g("fc1m", [C, HID])
    uneg = g("uneg", [1, HID])
    gbias = g("gbias", [128, NMO], F32)
    fc2w = g("fc2w", [HID, C])
    fc2b = g("fc2b", [128, NCC], F32)
    g1f = g("g1f", [128, NCC], F32); be1f = g("be1f", [128, NCC], F32)
    ones1 = g("ones1", [1, 128])
    ones128 = g("ones128", [128, 1])

    out = nc.dram_tensor("out", [C, TSH], F32, kind="ExternalOutput")
    rg = [list(range(NCORES))]

    from contextlib import ExitStack
    with tile.TileContext(nc) as tc:
        with ExitStack() as st0:
            cp = st0.enter_context(tc.tile_pool(name="const", bufs=1))
            dram = st0.enter_context(tc.tile_pool(name="dram", bufs=1, space="DRAM"))

            def cl(t, shape, dt=BF16):
                nm = f"c_{t.name}"
                s = cp.tile(shape, dt, name=nm, tag=nm)
                nc.sync.dma_start(s[:], t[:])
                return s

            c_fwp0 = cl(fwp0, [90, 92]); c_fwp1 = cl(fwp1, [90, 92])
            c_f2a = cl(f2a, [90, 180]); c_f2b = cl(f2b, [90, 180])
            c_iha = cl(iha, [90, 180]); c_ihb = cl(ihb, [90, 180])
            c_iwrt = cl(iwrt, [KW, W]); c_iwit = cl(iwit, [KW, W])
            c_w1r = cl(w1r, [BS, BS]); c_w1i = cl(w1i, [BS, BS])
            c_w1im = cl(w1im, [BS, BS])
            c_b1r = cl(b1r, [BS, 1], F32); c_b1i = cl(b1i, [BS, 1], F32)
            c_b1sr = cl(b1sr, [BS, 1], F32); c_b1si = cl(b1si, [BS, 1], F32)
            c_w2p1 = cl(w2p1, [BS, 192]); c_w2p2 = cl(w2p2, [BS, 192])
            c_b2pk = cl(b2pk, [1, 192])
            c_gbias = cl(gbias, [128, NMO], F32)
            c_fc2b = cl(fc2b, [128, NCC], F32)
            c_g1f = cl(g1f, [128, NCC], F32); c_be1f = cl(be1f, [128, NCC], F32)
            c_uneg = cl(uneg, [1, HID])
            c_ones1 = cl(ones1, [1, 128]); c_ones128 = cl(ones128, [128, 1])
            c_eps = cp.tile([128, 1], F32, name="c_eps")
            nc.vector.memset(c_eps[:], EPS)

            st_in = [dram.tile([2, TOKB], F32, name=f"st_in{b_}") for b_ in range(B)]
            st_out = [dram.tile([2, TOKB], F32, name=f"st_out{b_}") for b_ in range(B)]
            a2a_in = [dram.tile([NCORES, BS, TSB], BF16, name=f"a2a_in{b_}")
                      for b_ in range(B)]
            a2a_out = [dram.tile([NCORES, BS, TSB], BF16, name=f"a2a_out{b_}")
                       for b_ in range(B)]
            rows_dram = dram.tile([B, 2, TSB], BF16, name="rows_dram")

            # ================= phase 1 =================
            with ExitStack() as st1:
                zp = st1.enter_context(tc.tile_pool(name="zp", bufs=2))
                sqp = st1.enter_context(tc.tile_pool(name="sqp", bufs=2))
                clp = st1.enter_context(tc.tile_pool(name="clp", bufs=2))
                stp = st1.enter_context(tc.tile_pool(name="stats", bufs=1))
                ybo2 = st1.enter_context(tc.tile_pool(name="ybo2", bufs=2))
                zbp = st1.enter_context(tc.tile_pool(name="zbp", bufs=1))
                o1p = st1.enter_context(tc.tile_pool(name="o1p", bufs=1))
                u2p = st1.enter_context(tc.tile_pool(name="u2p", bufs=1))
                s2p = st1.enter_context(tc.tile_pool(name="s2p", bufs=1))
                pp = st1.enter_context(tc.tile_pool(name="psum1", bufs=8,
                                                    space="PSUM"))
                zhs = {}

                def stk(t, kind):
                    return bass.AP(tensor=t[:].tensor,
                                   offset=t[:].offset + kind * TOKB,
                                   ap=[[90, 90], [8100, 2], [1, 90]])

                def emit_loads_stats(b):
                    """Load z (bf16), partial LN1 stats, AllReduce trigger."""
                    eng = nc.vector
                    zh = []
                    for wc in range(2):
                        zt = zp.tile([90, H, BS], BF16, tag="zh",
                                     name=f"zh{b}{wc}")
                        eng_ld = nc.scalar if b == 0 else nc.sync
                        eng_ld.dma_start(
                            zt[:], xw[wc * 90:(wc + 1) * 90, b, :, :])
                        zh.append(zt)
                    zhs[b] = zh
                    s_sum = stp.tile([90, 2, H], F32, tag="ssum")
                    s_sq = stp.tile([90, 2, H], F32, tag="ssq")
                    s_t = stp.tile([90, H], F32, tag="st_t")
                    zhs[b, "sum"] = s_sum
                    zhs[b, "sq"] = s_sq
                    for wc in range(2):
                        zt = zh[wc]
                        eng.reduce_sum(s_sum[:, wc, :], zt[:], axis=AX.X)
                        # squared sums in 24-channel blocks (small scratch)
                        for blk in range(4):
                            sqt = sqp.tile([90, H, 24], BF16, tag="sqt")
                            zsl = zt[:, :, blk * 24:(blk + 1) * 24]
                            if b == 0:
                                nc.scalar.activation(out=sqt[:], in_=zsl,
                                                     func=AF.Square)
                            else:
                                nc.gpsimd.tensor_mul(sqt[:], zsl, zsl)
                            if blk == 0:
                                eng.reduce_sum(s_sq[:, wc, :], sqt[:], axis=AX.X)
                            else:
                                eng.reduce_sum(s_t[:], sqt[:], axis=AX.X)
                                eng.tensor_add(s_sq[:, wc, :], s_sq[:, wc, :],
                                               s_t[:])
                    nc.sync.dma_start(stk(st_in[b], 0), s_sum[:])
                    nc.sync.dma_start(stk(st_in[b], 1), s_sq[:])
                    nc.gpsimd.collective_compute(
                        "AllReduce", OP.add, replica_groups=rg,
                        ins=[st_in[b][:].opt()], outs=[st_out[b][:].opt()])

                def emit_post_stats(b):
                    """st recv, m/r, phase-2 rows, LN1 apply in place."""
                    s_sum, s_sq = zhs[b, "sum"], zhs[b, "sq"]
                    nc.sync.dma_start(s_sum[:], stk(st_out[b], 0))
                    nc.sync.dma_start(s_sq[:], stk(st_out[b], 1))
                    s_m = stp.tile([90, 2, H], F32, tag="sm")
                    s_r = stp.tile([90, 2, H], F32, tag="sr")
                    s_v = stp.tile([90, 2, H], F32, tag="sv")
                    nc.vector.tensor_scalar(out=s_m[:], in0=s_sum[:],
                                            scalar1=1.0 / C, scalar2=None,
                                            op0=OP.mult)
                    nc.vector.tensor_scalar(out=s_r[:], in0=s_sq[:],
                                            scalar1=1.0 / C, scalar2=None,
                                            op0=OP.mult)
                    nc.vector.tensor_mul(s_v[:], s_m[:], s_m[:])
                    nc.vector.tensor_sub(s_r[:], s_r[:], s_v[:])
                    nc.scalar.activation(out=s_r[:], in_=s_r[:],
                                         func=AF.Sqrt, bias=c_eps[:90])
                    nc.vector.reciprocal(s_r[:], s_r[:])
                    s_rb = stp.tile([90, 2, H], BF16, tag="srb")
                    s_mrb = stp.tile([90, 2, H], BF16, tag="smrb")
                    nc.vector.tensor_copy(s_rb[:], s_r[:])
                    nc.vector.tensor_mul(s_v[:], s_m[:], s_r[:])
                    nc.vector.tensor_copy(s_mrb[:], s_v[:])

                    for wc in range(2):
                        zt = zhs[b][wc]

                        def bc(t):
                            a = t[:, wc, :]
                            return bass.AP(tensor=a.tensor, offset=a.offset,
                                           ap=[list(a.ap[0]), [1, H], [0, BS]])
                        nc.vector.tensor_mul(zt[:], zt[:], bc(s_rb))
                        nc.vector.tensor_sub(zt[:], zt[:], bc(s_mrb))

                def emit_f1(b):
                    zh = zhs[b]
                    yb = ybo2.tile([90, BS, 92], BF16, tag="ybo2", name=f"yb{b}")
                    zhs[b, "yb"] = yb
                    for gi, c0 in enumerate(range(0, BS, 4)):
                        pf = pp.tile([90, 4 * 92], F32, tag="pp", name="psf1")
                        for ci in range(4):
                            c = c0 + ci
                            nc.tensor.matmul(pf[:, ci * 92:(ci + 1) * 92],
                                             zh[0][:, :, c], c_fwp0[:],
                                             start=True, stop=False)
                            nc.tensor.matmul(pf[:, ci * 92:(ci + 1) * 92],
                                             zh[1][:, :, c], c_fwp1[:],
                                             start=False, stop=True)
                        dst = yb[:, c0:c0 + 4, :]
                        src = pf[:].rearrange("p (a b) -> p a b", a=4)
                        if gi % 2 == 0:
                            nc.scalar.activation(out=dst, in_=src, func=AF.Copy)
                        else:
                            nc.vector.tensor_copy(dst, src)

                def emit_f2(b):
                    yb = zhs[b, "yb"]
                    # ---- F2 (flip, packed r/i)
                    zb = zbp.tile([BS, 2, KW, H], BF16, tag="zb", name=f"zb{b}")
                    zhs[b, "zb"] = zb
                    for wf in range(KW):
                        pz = pp.tile([BS, 180], F32, tag="pp", name="psf2")
                        nc.tensor.matmul(pz[:], yb[:, :, wf], c_f2a[:],
                                         start=True, stop=False)
                        nc.tensor.matmul(pz[:], yb[:, :, 46 + wf], c_f2b[:],
                                         start=False, stop=True)
                        nc.vector.tensor_copy(
                            zb[:, :, wf, :],
                            pz[:].rearrange("p (a b) -> p a b", a=2))

                def emit_rest(b):
                    zb = zhs[b, "zb"]
                    # ---- block MLP layer 1 (weight-stationary) + Relu
                    o1 = o1p.tile([BS, 2, NM], BF16, tag="o1", name=f"o1{b}")
                    zr_f = zb[:, 0].rearrange("p a b -> p (a b)")
                    zi_f = zb[:, 1].rearrange("p a b -> p (a b)")
                    for ch in range(9):
                        n0 = ch * M1CH
                        zr_s = zr_f[:, n0:n0 + M1CH]
                        zi_s = zi_f[:, n0:n0 + M1CH]
                        por = pp.tile([BS, M1CH], F32, tag="pp", name="pso1r")
                        nc.tensor.matmul(por[:], c_w1r[:], zr_s,
                                         start=True, stop=False)
                        nc.tensor.matmul(por[:], c_w1im[:], zi_s,
                                         start=False, stop=True)
                        poi = pp.tile([BS, M1CH], F32, tag="pp", name="pso1i")
                        nc.tensor.matmul(poi[:], c_w1i[:], zr_s,
                                         start=True, stop=False)
                        nc.tensor.matmul(poi[:], c_w1r[:], zi_s,
                                         start=False, stop=True)
                        if ch == 0:
                            # be1 spike contribution on mode (0,0) only
                            nc.vector.tensor_scalar(out=por[:, 0:1],
                                                    in0=por[:, 0:1],
                                                    scalar1=c_b1sr[:],
                                                    scalar2=None, op0=OP.add)
                            nc.vector.tensor_scalar(out=poi[:, 0:1],
                                                    in0=poi[:, 0:1],
                                                    scalar1=c_b1si[:],
                                                    scalar2=None, op0=OP.add)
                        nc.scalar.activation(out=o1[:, 0, n0:n0 + M1CH],
                                             in_=por[:], func=AF.Relu,
                                             bias=c_b1r[:])
                        nc.scalar.activation(out=o1[:, 1, n0:n0 + M1CH],
                                             in_=poi[:], func=AF.Relu,
                                             bias=c_b1i[:])

                    # ---- block MLP layer 2 (flip, packed) + softshrink
                    o2 = ybo2.tile([H, 2, KW, BS], BF16, tag="ybo2",
                                   name=f"o2{b}")
                    for wf in range(KW):
                        lr = o1[:, 0, wf * H:(wf + 1) * H]
                        li = o1[:, 1, wf * H:(wf + 1) * H]
                        pm = pp.tile([H, 192], F32, tag="pp", name="pso2")
                        nc.tensor.matmul(pm[:], lr, c_w2p1[:],
                                         start=True, stop=False)
                        nc.tensor.matmul(pm[:], li, c_w2p2[:],
                                         start=False, stop=False)
                        nc.tensor.matmul(pm[:], c_ones1[:, 0:H], c_b2pk[:],
                                         start=False, stop=True)
                        clip = clp.tile([H, 192], F32, tag="clip")
                        nc.vector.tensor_scalar(out=clip[:], in0=pm[:],
                                                scalar1=-LAM, scalar2=LAM,
                                                op0=OP.max, op1=OP.min)
                        nc.vector.tensor_tensor(
                            out=o2[:, :, wf, :],
                            in0=pm[:].rearrange("p (a b) -> p a b", a=2),
                            in1=clip[:].rearrange("p (a b) -> p a b", a=2),
                            op=OP.subtract)

                    # ---- inverse H-DFT (flip, packed) -> u2 [46, 2, 96, 90]
                    u2 = u2p.tile([KW, 2, BS, H], BF16, tag="u2", name=f"u2{b}")
                    for c in range(BS):
                        lr = o2[:, 0, :, c]
                        li = o2[:, 1, :, c]
                        pu = pp.tile([KW, 180], F32, tag="pp", name="psu")
                        nc.tensor.matmul(pu[:], lr, c_iha[:],
                                         start=True, stop=False)
                        nc.tensor.matmul(pu[:], li, c_ihb[:],
                                         start=False, stop=True)
                        dst = u2[:, :, c, :]
                        src = pu[:].rearrange("p (a b) -> p a b", a=2)
                        if c % 2 == 0:
                            nc.scalar.activation(out=dst, in_=src, func=AF.Copy)
                        else:
                            nc.vector.tensor_copy(dst, src)

                    # ---- inverse W-DFT (flip) -> s2 [90h, 96c, 180w]
                    s2 = s2p.tile([H, BS, W], BF16, tag="s2", name=f"s2{b}")
                    for c in range(BS):
                        pf = pp.tile([H, W], F32, tag="pp", name="psw")
                        nc.tensor.matmul(pf[:], u2[:, 0, c, :], c_iwrt[:],
                                         start=True, stop=False)
                        nc.tensor.matmul(pf[:], u2[:, 1, c, :], c_iwit[:],
                                         start=False, stop=True)
                        if c % 2 == 0:
                            nc.scalar.activation(out=s2[:, c, :], in_=pf[:],
                                                 func=AF.Copy)
                        else:
                            nc.vector.tensor_copy(s2[:, c, :], pf[:])

                    # ---- a2a send pieces (SBUF -> DRAM, w-contiguous)
                    for j in range(NCORES):
                        t0 = TSB * j
                        for (h0, h1, w0, w1) in _send_pieces(j):
                            src = s2[h0:h1, :, w0:w1]
                            dst = bass.AP(
                                tensor=a2a_in[b][:].tensor,
                                offset=(a2a_in[b][:].offset
                                        + j * BS * TSB
                                        + (h0 * W + w0 - t0)),
                                ap=[[W, h1 - h0], [TSB, BS], [1, w1 - w0]])
                            nc.sync.dma_start(dst, src)

                # emission order chosen so collective triggers never block
                # earlier-needed work on the same engine queue
                upf = st1.enter_context(tc.tile_pool(name="upf", bufs=8))
                upr = st1.enter_context(tc.tile_pool(name="upr", bufs=1))

                def emit_p2rows(tb):
                    """Phase-2 LN1 rows from xc (upfront; also warms the PE).
                    tb=0 squares on DVE (early, idle); tb=1 on gpsimd."""
                    for tt in range(NT):
                        tg = tb * TSB + tt * TT
                        xus = []
                        for cc in range(NCC):
                            xu = upf.tile([128, TT], BF16, tag="xu", name="xu")
                            nc.gpsimd.dma_start(
                                xu[:], xc[cc * 128:(cc + 1) * 128, tg:tg + TT])
                            xus.append(xu)
                        ps_m = pp.tile([1, TT], F32, tag="pp", name="psmu")
                        ps_q = pp.tile([1, TT], F32, tag="pp", name="psqu")
                        for cc in range(NCC):
                            nc.tensor.matmul(ps_m[:], c_ones128[:], xus[cc][:],
                                             start=(cc == 0),
                                             stop=(cc == NCC - 1))
                        for cc in range(NCC):
                            xq = upf.tile([128, TT], BF16, tag="xu", name="xq")
                            if tb == 0:
                                nc.vector.tensor_mul(xq[:], xus[cc][:],
                                                     xus[cc][:])
                            else:
                                nc.gpsimd.tensor_mul(xq[:], xus[cc][:],
                                                     xus[cc][:])
                            nc.tensor.matmul(ps_q[:], c_ones128[:], xq[:],
                                             start=(cc == 0),
                                             stop=(cc == NCC - 1))
                        m1 = upr.tile([1, TT], F32, tag="m1u")
                        r1 = upr.tile([1, TT], F32, tag="r1u")
                        v1 = upr.tile([1, TT], F32, tag="v1u")
                        nc.vector.tensor_scalar(out=m1[:], in0=ps_m[:],
                                                scalar1=1.0 / C, scalar2=None,
                                                op0=OP.mult)
                        nc.vector.tensor_scalar(out=r1[:], in0=ps_q[:],
                                                scalar1=1.0 / C, scalar2=None,
                                                op0=OP.mult)
                        nc.vector.tensor_mul(v1[:], m1[:], m1[:])
                        nc.vector.tensor_sub(r1[:], r1[:], v1[:])
                        nc.scalar.activation(out=r1[:], in_=r1[:],
                                             func=AF.Sqrt, bias=c_eps[:1])
                        nc.vector.reciprocal(r1[:], r1[:])
                        rb1 = upr.tile([1, TT], BF16, tag="rb1u")
                        rb2 = upr.tile([1, TT], BF16, tag="rb2u")
                        nc.vector.tensor_copy(rb1[:], r1[:])
                        nc.vector.tensor_mul(m1[:], m1[:], r1[:])
                        nc.vector.tensor_copy(rb2[:], m1[:])
                        nc.sync.dma_start(
                            rows_dram[tb, 0, tt * TT:(tt + 1) * TT], rb1[:])
                        nc.sync.dma_start(
                            rows_dram[tb, 1, tt * TT:(tt + 1) * TT], rb2[:])

                emit_p2rows(0)
                emit_loads_stats(0)
                emit_post_stats(0)
                emit_f1(0)
                emit_f2(0)
                emit_loads_stats(1)     # AR1 triggers before a2a_0
                emit_p2rows(1)
                emit_rest(0)
                nc.gpsimd.collective_compute(
                    "AllToAll", OP.bypass, replica_groups=rg,
                    ins=[a2a_in[0][:].opt()], outs=[a2a_out[0][:].opt()])
                emit_post_stats(1)
                emit_f1(1)
                emit_f2(1)
                emit_rest(1)

            # ================= phase 2 =================
            with ExitStack() as st2:
                fc1p = st2.enter_context(tc.tile_pool(name="fc1p", bufs=1))
                fc2p = st2.enter_context(tc.tile_pool(name="fc2p", bufs=1))
                xtp = st2.enter_context(tc.tile_pool(name="xtp", bufs=12))
                hrp = st2.enter_context(tc.tile_pool(name="hrp", bufs=12))
                hbp = st2.enter_context(tc.tile_pool(name="hbp", bufs=12))
                hip = st2.enter_context(tc.tile_pool(name="hip", bufs=1))
                t1p = st2.enter_context(tc.tile_pool(name="t1p", bufs=2))
                hqp = st2.enter_context(tc.tile_pool(name="hqp", bufs=6))
                hcp = st2.enter_context(tc.tile_pool(name="hcp", bufs=12))
                rwp = st2.enter_context(tc.tile_pool(name="rwp", bufs=4))
                rbp = st2.enter_context(tc.tile_pool(name="rbp", bufs=6))
                rw2 = st2.enter_context(tc.tile_pool(name="rw2", bufs=1))
                outp = st2.enter_context(tc.tile_pool(name="outp", bufs=2))
                ph = st2.enter_context(tc.tile_pool(name="ph", bufs=2, space="PSUM"))
                po = st2.enter_context(tc.tile_pool(name="po", bufs=2, space="PSUM"))
                pst = st2.enter_context(tc.tile_pool(name="pst", bufs=2, space="PSUM"))
                pbc = st2.enter_context(tc.tile_pool(name="pbc", bufs=2, space="PSUM"))

                c_fc1 = [fc1p.tile([128, HID], BF16, tag=f"fc1_{i}", name=f"cfc1_{i}")
                         for i in range(NCC)]
                for i in range(NCC):
                    nc.gpsimd.dma_start(c_fc1[i][:], fc1m[i * 128:(i + 1) * 128, :])
                c_fc2 = [fc2p.tile([128, C], BF16, tag=f"fc2_{i}", name=f"cfc2_{i}")
                         for i in range(NMO)]
                for i in range(NMO):
                    nc.gpsimd.dma_start(c_fc2[i][:], fc2w[i * 128:(i + 1) * 128, :])

                def p2_prep(tb, tt):
                    t0 = tt * TT
                    tg = tb * TSB + t0
                    r1w = rwp.tile([1, TT], BF16, tag="r1w", name="r1w")
                    mr1w = rwp.tile([1, TT], BF16, tag="mr1w", name="mr1w")
                    nc.sync.dma_start(r1w[:], rows_dram[tb, 0, t0:t0 + TT])
                    nc.sync.dma_start(mr1w[:], rows_dram[tb, 1, t0:t0 + TT])
                    r1b = rbp.tile([128, TT], BF16, tag="r1b", name="r1b")
                    mr1b = rbp.tile([128, TT], BF16, tag="mr1b", name="mr1b")
                    for rows, bt in ((r1w, r1b), (mr1w, mr1b)):
                        pb = pbc.tile([128, TT], F32, tag="pbc", name="pbt")
                        nc.tensor.matmul(pb[:], c_ones1[:], rows[:],
                                         start=True, stop=True)
                        nc.vector.tensor_copy(bt[:], pb[:])
                    htbs, hsqs = [], []
                    for cc in range(NCC):
                        xct = xtp.tile([128, TT], BF16, tag="xct", name="xct")
                        nc.gpsimd.dma_start(
                            xct[:], xc[cc * 128:(cc + 1) * 128, tg:tg + TT])
                        htr = hrp.tile([128, TT], BF16, tag="htr", name="htr")
                        for (r0, sc, ci, n) in _recv_pieces(cc):
                            nc.gpsimd.dma_start(
                                htr[r0:r0 + n, :],
                                a2a_out[tb][sc, ci:ci + n, t0:t0 + TT])
                        t1 = t1p.tile([128, TT], BF16, tag="t1", name="t1")
                        nc.vector.tensor_mul(t1[:], xct[:], r1b[:])
                        nc.vector.tensor_sub(t1[:], t1[:], mr1b[:])
                        nc.vector.tensor_scalar(out=t1[:], in0=t1[:],
                                                scalar1=c_g1f[:, cc:cc + 1],
                                                scalar2=c_be1f[:, cc:cc + 1],
                                                op0=OP.mult, op1=OP.add)
                        htb = hbp.tile([128, TT], BF16, tag="htb", name="htb")
                        nc.vector.tensor_add(htb[:], htr[:], xct[:])
                        nc.vector.tensor_add(htb[:], htb[:], t1[:])
                        htbs.append(htb)
                        hsq = hqp.tile([128, TT], BF16, tag="hsq", name="hsq")
                        nc.vector.tensor_mul(hsq[:], htb[:], htb[:])
                        hsqs.append(hsq)
                    return htbs, hsqs

                def p2_ln2(htbs, hsqs):
                    ps_s = pst.tile([1, TT], F32, tag="pst", name="pss")
                    ps_q = pst.tile([1, TT], F32, tag="pst", name="psq")
                    for cc in range(NCC):
                        nc.tensor.matmul(ps_s[:], c_ones128[:], htbs[cc][:],
                                         start=(cc == 0), stop=(cc == NCC - 1))
                    for cc in range(NCC):
                        nc.tensor.matmul(ps_q[:], c_ones128[:], hsqs[cc][:],
                                         start=(cc == 0), stop=(cc == NCC - 1))
                    m2r = rw2.tile([1, TT], F32, tag="m2r", name="m2r")
                    r2r = rw2.tile([1, TT], F32, tag="r2r", name="r2r")
                    vv = rw2.tile([1, TT], F32, tag="vv", name="vv")
                    nc.vector.tensor_scalar(out=m2r[:], in0=ps_s[:],
                                            scalar1=1.0 / C, scalar2=None,
                                            op0=OP.mult)
                    nc.vector.tensor_scalar(out=r2r[:], in0=ps_q[:],
                                            scalar1=1.0 / C, scalar2=None,
                                            op0=OP.mult)
                    nc.vector.tensor_mul(vv[:], m2r[:], m2r[:])
                    nc.vector.tensor_sub(r2r[:], r2r[:], vv[:])
                    nc.scalar.activation(out=r2r[:], in_=r2r[:],
                                         func=AF.Sqrt, bias=c_eps[:1])
                    nc.vector.reciprocal(r2r[:], r2r[:])
                    r2bf = rw2.tile([1, TT], BF16, tag="r2bf", name="r2bf")
                    m2rb = rw2.tile([1, TT], BF16, tag="m2rb", name="m2rb")
                    nc.vector.tensor_copy(r2bf[:], r2r[:])
                    nc.vector.tensor_mul(m2r[:], m2r[:], r2r[:])
                    nc.vector.tensor_copy(m2rb[:], m2r[:])
                    r2s = rbp.tile([128, TT], BF16, tag="r2s", name="r2s")
                    pb2 = pbc.tile([128, TT], F32, tag="pbc", name="pb2")
                    nc.tensor.matmul(pb2[:], c_ones1[:], r2bf[:],
                                     start=True, stop=True)
                    nc.vector.tensor_copy(r2s[:], pb2[:])
                    htcs = []
                    for cc in range(NCC):
                        htc = hcp.tile([128, TT], BF16, tag="htc", name="htc")
                        nc.vector.tensor_mul(htc[:], htbs[cc][:], r2s[:])
                        htcs.append(htc)
                    return htcs, m2rb

                def p2_compute(tb, tt, htbs, htcs, m2rb):
                    tg = tb * TSB + tt * TT
                    hid = hip.tile([128, NMO, TT], BF16, tag="hid", name="hid")
                    for mo in range(NMO):
                        php = ph.tile([128, TT], F32, tag="ph", name="php")
                        for cc in range(NCC):
                            nc.tensor.matmul(
                                php[:],
                                c_fc1[cc][:, mo * 128:(mo + 1) * 128],
                                htcs[cc][:],
                                start=(cc == 0), stop=False)
                        nc.tensor.matmul(php[:],
                                         c_uneg[:, mo * 128:(mo + 1) * 128],
                                         m2rb[:], start=False, stop=True)
                        nc.scalar.activation(out=hid[:, mo, :], in_=php[:],
                                             func=AF.Gelu,
                                             bias=c_gbias[:, mo:mo + 1])
                    for co in range(NCC):
                        pop = po.tile([128, TT], F32, tag="po", name="pop")
                        for ho in range(NMO):
                            nc.tensor.matmul(
                                pop[:],
                                c_fc2[ho][:, co * 128:(co + 1) * 128],
                                hid[:, ho, :],
                                start=(ho == 0), stop=(ho == NMO - 1))
                        of = outp.tile([128, TT], F32, tag="of", name="of")
                        nc.vector.scalar_tensor_tensor(
                            out=of[:], in0=pop[:],
                            scalar=c_fc2b[:, co:co + 1], in1=htbs[co][:],
                            op0=OP.add, op1=OP.add)
                        nc.sync.dma_start(
                            out[co * 128:(co + 1) * 128, tg:tg + TT], of[:])

                for tb in range(B):
                    cur = p2_prep(tb, 0)
                    for tt in range(NT):
                        htbs, hsqs = cur
                        htcs, m2rb = p2_ln2(htbs, hsqs)
                        if tt + 1 < NT:
                            cur = p2_prep(tb, tt + 1)
                        p2_compute(tb, tt, htbs, htcs, m2rb)
                    if tb == 0:
                        # emitted after tb=0 recv DMAs so its trigger wait
                        # (b=1 sends) never delays them on the gpsimd queue
                        nc.gpsimd.collective_compute(
                            "AllToAll", OP.bypass, replica_groups=rg,
                            ins=[a2a_in[1][:].opt()],
                            outs=[a2a_out[1][:].opt()])

    nc.compile()
    _CACHE["nc"] = nc
    return nc


def _host_prep(inputs):
    x = np.ascontiguousarray(np.asarray(inputs["x"], dtype=np.float32))
    g1 = np.asarray(inputs["g1"], np.float32); be1 = np.asarray(inputs["be1"], np.float32)
    g2 = np.asarray(inputs["g2"], np.float32); be2 = np.asarray(inputs["be2"], np.float32)
    w1 = np.asarray(inputs["w1"], np.float32); b1 = np.asarray(inputs["b1"], np.float32)
    w2 = np.asarray(inputs["w2"], np.float32); b2 = np.asarray(inputs["b2"], np.float32)
    fc1_w = np.asarray(inputs["fc1_w"], np.float32)
    fc1_b = np.asarray(inputs["fc1_b"], np.float32)
    fc2_w = np.asarray(inputs["fc2_w"], np.float32)
    fc2_b = np.asarray(inputs["fc2_b"], np.float32)

    dft = _dft_consts()
    xf = x.reshape(B, TOKB, C)
    fc1s = g2[:, None] * fc1_w
    fc1m_m = fc1s.astype(BF)                                     # (768, 3072)
    uneg_m = (-fc1s.sum(0, dtype=np.float64)).astype(BF)[None, :]
    gbias_v = (fc1_b + be2 @ fc1_w).astype(np.float32)           # (3072,)
    gbias_m = np.ascontiguousarray(gbias_v.reshape(NMO, 128).T)  # (128, 24)
    fc2b_m = np.ascontiguousarray(fc2_b.reshape(NCC, 128).T)
    g1f_m = np.ascontiguousarray(g1.reshape(NCC, 128).T)
    be1f_m = np.ascontiguousarray(be1.reshape(NCC, 128).T)
    ones1 = np.ones((1, 128), BF)
    ones128 = np.ones((128, 1), BF)

    in_maps = []
    for k in range(NCORES):
        ck = slice(k * BS, (k + 1) * BS)
        xw_k = np.ascontiguousarray(
            x[:, :, :, ck].transpose(2, 0, 1, 3)).astype(BF)     # [W,B,H,BS]
        xc_k = np.ascontiguousarray(
            np.concatenate([xf[0, k * TSB:(k + 1) * TSB],
                            xf[1, k * TSB:(k + 1) * TSB]], 0).T).astype(BF)
        g1k = g1[ck]
        w1r_k = w1[k, :, :, 0]; w1i_k = w1[k, :, :, 1]
        w1rp = (g1k[:, None] * w1r_k).astype(BF)
        w1ip = (g1k[:, None] * w1i_k).astype(BF)
        w1imp = (-(g1k[:, None] * w1i_k)).astype(BF)
        spike = (be1[ck] * SQN).astype(np.float64)
        b1sr_k = (w1r_k.T.astype(np.float64) @ spike).astype(np.float32)[:, None]
        b1si_k = (w1i_k.T.astype(np.float64) @ spike).astype(np.float32)[:, None]
        w2r_k = w2[k, :, :, 0]; w2i_k = w2[k, :, :, 1]
        w2p1_k = np.concatenate([w2r_k, w2i_k], 1).astype(BF)    # [96, 192]
        w2p2_k = np.concatenate([-w2i_k, w2r_k], 1).astype(BF)
        b2pk_k = np.concatenate([b2[k, :, 0], b2[k, :, 1]])[None, :].astype(BF)
        in_maps.append({
            "xw": xw_k, "xc": xc_k,
            **{n: dft[n] for n in ("fwp0", "fwp1", "f2a", "f2b", "iha", "ihb",
                                   "iwrt", "iwit")},
            "w1r": w1rp, "w1i": w1ip, "w1im": w1imp,
            "b1r": b1[k, :, 0:1].copy(), "b1i": b1[k, :, 1:2].copy(),
            "b1sr": b1sr_k, "b1si": b1si_k,
            "w2p1": w2p1_k, "w2p2": w2p2_k, "b2pk": b2pk_k,
            "fc1m": fc1m_m, "uneg": uneg_m, "gbias": gbias_m,
            "fc2w": fc2_w.astype(BF), "fc2b": fc2b_m,
            "g1f": g1f_m, "be1f": be1f_m,
            "ones1": ones1, "ones128": ones128,
        })
    return in_maps


def kernel(**inputs):
    nc = _build_nc()
    in_maps = _host_prep(inputs)
    res = run_bass_kernel_spmd(nc, in_maps, core_ids=list(range(NCORES)))
    full = np.empty((B, TOKB, C), np.float32)
    for j in range(NCORES):
        o = np.asarray(res.results[j]["out"], np.float32).T   # [4050, 768]
        full[0, j * TSB:(j + 1) * TSB] = o[:TSB]
        full[1, j * TSB:(j + 1) * TSB] = o[TSB:]
    return full.reshape(B, H, W, C)
